# revision 30
# baseline (speedup 1.0000x reference)
"""Multi-head attention + LayerNorm Trainium2 kernel (v3).

Full inputs: x [8, 1024, 512], Wq/Wk/Wv [512, 512], ln_gamma/ln_beta [512].
Data-parallel over batch: one batch element per NeuronCore (8 cores), no
collectives. Host preprocessing ships transposed bf16 views of the inputs
(xT [E,S], WqT/WkT/WvT [E,E]) so the device does no layout transposes.

Per-core dataflow (S=1024, E=512, H=8 heads, D=64 head dim):
  1. Projections q^T/k^T in [e_out, s] layout (bf16 operands, f32 PSUM).
     DVE quantizes q to fp8e4m3 as a (hi, residual) pair and k to a
     duplicated fp8 pair, enabling DoubleRow QK matmuls: the pair dim
     contracts (k,k)x(q_hi,q_res) = k·(q_hi+q_res), i.e. q at ~bf16
     precision, k at fp8, 0.5 cycles/output column.
  2. scores^T [sk, sq] per (head, sk-tile); exp with the 1/sqrt(E) scale
     fused, bf16 out, split between ScalarE (activation Exp) and the
     otherwise-idle GPSIMD engine (tensor_tensor pow: e^(s*x) = b^x with
     b = e^s, measured exact to ~2e-6 on hw). No max subtraction needed:
     scores are ~N(0, 0.35).
  3. AV in [sq, d] orientation: out[sq-128, D+1] accumulates over sk
     chunks with lhsT = exp tile (free dim = sq chunk), rhs = [v | 1]
     so the softmax normalizer lands in column D as a per-partition
     scalar. Output free size is 65, so this is ~2x cheaper on the PE
     than the [d, sq] orientation and needs no output transpose.
  4. Per (h, tq): GPSIMD divides cols 0..D by col D into o_all (keeps
     the PSUM accumulator ring off the in-order DVE queue), DVE
     bn_stats; LayerNorm + DMA out at the tail.
"""

import math
import numpy as np
import ml_dtypes
from contextlib import ExitStack

import concourse.bass as bass
import concourse.tile as tile
from concourse import bacc, mybir
from concourse.bass_utils import run_bass_kernel_spmd

S = 1024
E = 512
H = 8
D = 64
P = 128
NE = E // P   # 4 e-chunks
NS = S // P   # 8 s-tiles
DP1 = D + 1   # head dim + normalizer column
PH = DP1      # per-head stride in vext
SCALE = float(E) ** -0.5
BASE = math.exp(SCALE)
EPS = 1e-5

F32 = mybir.dt.float32
F32R = mybir.dt.float32r
BF16 = mybir.dt.bfloat16
FP8 = mybir.dt.float8e4
AF = mybir.ActivationFunctionType
ALU = mybir.AluOpType
DR = mybir.MatmulPerfMode.DoubleRow

QK_FP8 = True   # DoubleRow fp8 QK (k fp8, q hi+res fp8 pair)
K_BCAST = True  # stride-0 broadcast of k in the DoubleRow pair (no dup copy)
# exp tiles relayed (DVE copy to SBUF, then GPSIMD pow) off ScalarE:
# (head, sk-tile) pairs.  GPSIMD cannot read PSUM, hence the relay.
EXP_POOL = {(h, tk) for h in range(2, H) for tk in (2, 5)}


def _emit(nc, tc, xT_d, wqT_d, wkT_d, wvT_d, g_d, b_d, out_d, apply_gb):
    ctx = ExitStack()
    with ctx:
        persist = ctx.enter_context(tc.tile_pool(name="persist", bufs=1))
        ps_pool = ctx.enter_context(tc.tile_pool(name="ps", bufs=2, space="PSUM"))
        expp = ctx.enter_context(tc.tile_pool(name="expp", bufs=16))
        finp = ctx.enter_context(tc.tile_pool(name="fin", bufs=4))

        eps_t = persist.tile([P, 1], F32, tag="eps", name="eps")
        nc.vector.memset(eps_t, EPS)
        base_t = persist.tile([P, 1], F32, tag="base", name="base")
        nc.vector.memset(base_t, BASE)
        scr = persist.tile([P, 1], F32, tag="scr", name="scr")
        if apply_gb:
            gam_b = persist.tile([P, E], F32, tag="gam", name="gam")
            nc.gpsimd.dma_start(out=gam_b, in_=g_d.partition_broadcast(P))
            bet_b = persist.tile([P, E], F32, tag="bet", name="bet")
            nc.gpsimd.dma_start(out=bet_b, in_=b_d.partition_broadcast(P))

        xT = persist.tile([P, NE, S], BF16, tag="xT", name="xT")
        wT = persist.tile([P, 3, NE, E], BF16, tag="wT", name="wT")
        if QK_FP8:
            q8 = persist.tile([P, NE, 2, S], FP8, tag="q8", name="q8")
            k8 = persist.tile([P, NE, 2, S], FP8, tag="k8", name="k8")
        else:
            qT = persist.tile([P, NE, S], BF16, tag="q8", name="qT")
            kT = persist.tile([P, NE, S], BF16, tag="k8", name="kT")
        vext = persist.tile([P, NS, H * PH], BF16, tag="vext", name="vext")
        o_all = persist.tile([P, NS, E], F32, tag="o_all", name="o_all")
        st_all = persist.tile([P, NS, H, 6], F32, tag="st_all", name="st_all")

        # loads, priority-ordered for the first QK tiles: x half 0, then the
        # chunk-0 column slices of Wq/Wk (all the first head pair needs),
        # x half 1, the Wq/Wk remainders, Wv.
        def dma_x_half(n):
            nc.sync.dma_start(
                out=xT[:, :, n * 512:(n + 1) * 512],
                in_=xT_d.rearrange("(c p) s -> p c s", p=P)[:, :, n * 512:(n + 1) * 512],
            )

        def dma_w(wi, w_d, lo, hi):
            nc.sync.dma_start(
                out=wT[:, wi, :, lo:hi],
                in_=w_d.rearrange("(c p) e -> p c e", p=P)[:, :, lo:hi],
            )

        dma_x_half(0)
        dma_w(0, wqT_d, 0, P)
        dma_w(1, wkT_d, 0, P)
        dma_x_half(1)
        dma_w(0, wqT_d, P, E)
        dma_w(1, wkT_d, P, E)
        dma_w(2, wvT_d, 0, E)

        def proj_half(wi, c, n):
            """q/k chunk c, s-half n: psum [P, 512] so its preps only wait
            on this half's accumulation group.  Own 1-bank tag so projection
            tiles never block the score-tile ring (which would starve the
            exp stream)."""
            pp = ps_pool.tile([P, 512], F32, tag="pp", bufs=1,
                              name=f"pp{wi}_{c}_{n}")
            for ce in range(NE):
                nc.tensor.matmul(
                    out=pp,
                    lhsT=wT[:, wi, ce, c * P:(c + 1) * P],
                    rhs=xT[:, ce, n * 512:(n + 1) * 512],
                    start=(ce == 0), stop=(ce == NE - 1),
                )
            return pp

        def prep_q(c, n, pp):
            sl = slice(n * 512, (n + 1) * 512)
            if QK_FP8:
                nc.vector.tensor_copy(out=q8[:, c, 0, sl], in_=pp)
                nc.vector.tensor_tensor(
                    out=q8[:, c, 1, sl], in0=pp, in1=q8[:, c, 0, sl],
                    op=ALU.subtract,
                )
            else:
                nc.vector.tensor_copy(out=qT[:, c, sl], in_=pp)

        def prep_k(c, n, pp):
            sl = slice(n * 512, (n + 1) * 512)
            if QK_FP8:
                nc.vector.tensor_copy(out=k8[:, c, 0, sl], in_=pp)
                if not K_BCAST:
                    nc.gpsimd.tensor_copy(out=k8[:, c, 1, sl],
                                          in_=k8[:, c, 0, sl])
            else:
                nc.vector.tensor_copy(out=kT[:, c, sl], in_=pp)

        def prep_qk_chunk(c):
            for n in range(2):
                prep_q(c, n, proj_half(0, c, n))
                prep_k(c, n, proj_half(1, c, n))

        def vproj(t_i):
            pv = ps_pool.tile([P, E], F32, tag="pp", bufs=1, name=f"pv{t_i}")
            for ce in range(NE):
                nc.tensor.matmul(
                    out=pv,
                    lhsT=xT[:, ce, t_i * P:(t_i + 1) * P],
                    rhs=wT[:, 2, ce, :],
                    start=(ce == 0), stop=(ce == NE - 1),
                )
            vdst = vext[:, t_i, :].rearrange("p (h c) -> p h c", c=PH)[:, :, 0:D]
            nc.vector.tensor_copy(out=vdst, in_=pv.rearrange("p (h d) -> p h d", d=D))

        exp_tiles = {}

        def qk_mm(sp, h, tk, n):
            c, b = h // 2, D * (h % 2)
            if QK_FP8:
                if K_BCAST:
                    lhsT = k8[b:b + D, c, 0:1, tk * P:(tk + 1) * P] \
                        .broadcast_to([D, 2, P])
                else:
                    lhsT = k8[b:b + D, c, :, tk * P:(tk + 1) * P]
                nc.tensor.matmul(
                    out=sp[:, n * 512:(n + 1) * 512],
                    lhsT=lhsT,
                    rhs=q8[b:b + D, c, :, n * 512:(n + 1) * 512],
                    start=True, stop=True,
                    perf_mode=DR,
                )
            else:
                nc.tensor.matmul(
                    out=sp[:, n * 512:(n + 1) * 512],
                    lhsT=kT[b:b + D, c, tk * P:(tk + 1) * P],
                    rhs=qT[b:b + D, c, n * 512:(n + 1) * 512],
                    start=True, stop=True,
                )

        def exp_dst(h, tk):
            key = (h, tk // 2)
            if key not in exp_tiles:
                exp_tiles[key] = expp.tile([P, 2, S], BF16, tag="exp",
                                           name=f"e{h}_{tk}")
            return exp_tiles[key][:, tk % 2, :]

        def exp_half(sp, h, tk, n):
            sl = slice(n * 512, (n + 1) * 512)
            nc.scalar.activation(
                out=exp_dst(h, tk)[:, sl], in_=sp[:, sl], func=AF.Exp,
                scale=SCALE,
            )

        def qk(h, tk):
            """scores^T tile [sk=128, sq=1024] for head h, sk tile tk + exp."""
            sp = ps_pool.tile([P, S], F32, tag="ps", name=f"sc{h}_{tk}")
            qk_mm(sp, h, tk, 0)
            qk_mm(sp, h, tk, 1)
            dst = exp_dst(h, tk)
            if (h, tk) in EXP_POOL:
                stage = expp.tile([P, S], F32, tag="stage", bufs=3,
                                  name=f"st{h}_{tk}")
                for n in range(2):
                    sl = slice(n * 512, (n + 1) * 512)
                    nc.vector.tensor_copy(out=stage[:, sl], in_=sp[:, sl])
                nc.gpsimd.tensor_tensor(
                    out=dst, in0=base_t.broadcast_to([P, S]), in1=stage,
                    op=ALU.pow,
                )
            else:
                nc.scalar.activation(out=dst, in_=sp, func=AF.Exp, scale=SCALE)

        def av_head(h, tq, pu):
            """U[sq-tile tq, D+1] for head h: accumulate over all sk chunks."""
            for tk in range(NS):
                nc.tensor.matmul(
                    out=pu,
                    lhsT=exp_tiles[(h, tk // 2)][:, tk % 2, tq * P:(tq + 1) * P],
                    rhs=vext[:, tk, h * PH:h * PH + DP1],
                    start=(tk == 0), stop=(tk == NS - 1),
                )

        def fin_head(h, tq, pu, on_act=False):
            rc = finp.tile([P, 1], F32, tag="rc", name=f"rc{h}_{tq}")
            nc.vector.reciprocal(out=rc, in_=pu[:, D:DP1])
            if on_act:
                nc.scalar.activation(
                    out=o_all[:, tq, h * D:(h + 1) * D],
                    in_=pu[:, 0:D], func=AF.Copy, scale=rc,
                )
            else:
                nc.vector.tensor_scalar_mul(
                    out=o_all[:, tq, h * D:(h + 1) * D],
                    in0=pu[:, 0:D], scalar1=rc,
                )
            nc.vector.bn_stats(
                out=st_all[:, tq, h, :],
                in_=o_all[:, tq, h * D:(h + 1) * D],
            )

        def layer_norm(tq, xc_eng=None):
            mv = finp.tile([P, 2], F32, tag="mv", name=f"mv{tq}")
            nc.vector.bn_aggr(out=mv, in_=st_all[:, tq, :, :])
            sd = finp.tile([P, 1], F32, tag="sd", name=f"sd{tq}")
            nc.scalar.activation(out=sd, in_=mv[:, 1:2], func=AF.Sqrt, bias=eps_t)
            rs = finp.tile([P, 1], F32, tag="rs", name=f"rs{tq}")
            nc.vector.reciprocal(out=rs, in_=sd)
            xc = finp.tile([P, E], F32, tag="xc", name=f"xc{tq}")
            eng = xc_eng or nc.gpsimd
            eng.tensor_scalar(
                out=xc, in0=o_all[:, tq, :],
                scalar1=mv[:, 0:1], scalar2=rs,
                op0=ALU.subtract, op1=ALU.mult,
            )
            if apply_gb:
                eng.tensor_mul(out=xc, in0=xc, in1=gam_b)
                eng.tensor_add(out=xc, in0=xc, in1=bet_b)
            nc.sync.dma_start(out=out_d[tq * P:(tq + 1) * P, :], in_=xc)

        # AV accumulator ring: PSUM accumulation groups are bank-granular
        # (start_tensor_calc zeroes a whole 2KB bank), so each live
        # accumulator needs its own bank.  3 banks here; the last pair
        # reuses the score-tile banks for an 8-deep ring.
        pup = ps_pool.tile([P, 12, P], F32, tag="pu", bufs=1, name="pup")
        pu_ctr = [0]

        def pu_slot():
            s = 4 * (pu_ctr[0] % 3)
            pu_ctr[0] += 1
            return pup[:, s, 0:DP1]

        # ---- warmup: junk matmuls ramp the PE out of its low p-state while
        # the first DMAs are still in flight (targets the pup banks, which
        # see real accumulations only much later)
        wu = persist.tile([P, 640], BF16, tag="wu", name="wu")
        nc.gpsimd.memset(wu, 0.0)
        for i in range(8):
            wup = pup[:, 4 * (i % 3):4 * (i % 3) + 4, :] \
                .rearrange("p a b -> p (a b)")
            nc.tensor.matmul(out=wup, lhsT=wu[:, 0:P], rhs=wu[:, P:640],
                             start=True, stop=True)

        for t_i in range(NS):
            ones_v = vext[:, t_i, :].rearrange("p (h c) -> p h c", c=PH)[:, :, D:DP1]
            nc.gpsimd.memset(ones_v, 1.0)

        # ---- stage 1: q/k chunk 0; the first two score tiles run their
        # s-half 0 as soon as the half-0 preps land, with half-1 following.
        # v projection + the other q/k chunks ride the ScalarE-paced slack
        # one projection-half at a time so the score-tile ring never blocks.
        prep_q(0, 0, proj_half(0, 0, 0))
        prep_k(0, 0, proj_half(1, 0, 0))
        sp0 = ps_pool.tile([P, S], F32, tag="ps", name="sc0_0")
        qk_mm(sp0, 0, 0, 0)
        exp_half(sp0, 0, 0, 0)
        sp1 = ps_pool.tile([P, S], F32, tag="ps", name="sc0_1")
        qk_mm(sp1, 0, 1, 0)
        exp_half(sp1, 0, 1, 0)
        prep_q(0, 1, proj_half(0, 0, 1))
        prep_k(0, 1, proj_half(1, 0, 1))
        qk_mm(sp0, 0, 0, 1)
        exp_half(sp0, 0, 0, 1)
        qk_mm(sp1, 0, 1, 1)
        exp_half(sp1, 0, 1, 1)

        fillers = [lambda t_i=t_i: vproj(t_i) for t_i in range(NS)]
        for c in (1, 2, 3):
            for n in range(2):
                fillers.append(lambda c=c, n=n: prep_q(c, n, proj_half(0, c, n)))
                fillers.append(lambda c=c, n=n: prep_k(c, n, proj_half(1, c, n)))

        def filler():
            if fillers:
                fillers.pop(0)()

        for tk in range(2, NS):
            qk(0, tk)
            filler()
        for tk in range(NS):
            qk(1, tk)
            filler()

        # ---- stage 2: pair loops: AV of pair p, QK of pair p+1 ----------
        for p in range(H // 2 - 1):
            h0, h1 = 2 * p, 2 * p + 1
            for tq in range(NS):
                qk(2 * p + 2, tq)
                filler()
                qk(2 * p + 3, tq)
                pu0 = pu_slot()
                av_head(h0, tq, pu0)
                fin_head(h0, tq, pu0)
                pu1 = pu_slot()
                av_head(h1, tq, pu1)
                fin_head(h1, tq, pu1)

        # pre-switch the ACT table to the sqrt set now that the last exp has
        # been emitted, so the switch overlaps the final AV instead of the tail
        nc.scalar.activation(out=scr, in_=eps_t, func=AF.Sqrt)

        # last pair: no next-pair QK to interleave.  The score-tile banks
        # are free now, so rotate the AV accumulators over 8 PSUM banks —
        # the whole AV stream runs back-to-back on the PE while the fin /
        # LayerNorm chains drain behind it on DVE/ScalarE/GPSIMD.
        last_slots = [pup[:, 0, 0:DP1], pup[:, 4, 0:DP1], pup[:, 8, 0:DP1]]
        for i in range(2):
            t = ps_pool.tile([P, S], F32, tag="ps", name=f"fps{i}")
            last_slots += [t[:, 0:DP1], t[:, 512:512 + DP1]]
        t = ps_pool.tile([P, 512], F32, tag="pp", bufs=1, name="fpp")
        last_slots.append(t[:, 0:DP1])
        p = H // 2 - 1
        for tq in range(NS):
            pus = {}
            for h in (2 * p, 2 * p + 1):
                pus[h] = last_slots[(2 * tq + (h % 2)) % len(last_slots)]
                av_head(h, tq, pus[h])
            for h in (2 * p, 2 * p + 1):
                fin_head(h, tq, pus[h], on_act=(h % 2 == 0))
            layer_norm(tq, xc_eng=(nc.vector if tq % 2 else nc.gpsimd))


def build_attention(apply_gb=True):
    nc = bacc.Bacc("TRN2", target_bir_lowering=False, debug=False)
    xT_d = nc.dram_tensor("xT", [E, S], BF16, kind="ExternalInput").ap()
    wqT_d = nc.dram_tensor("WqT", [E, E], BF16, kind="ExternalInput").ap()
    wkT_d = nc.dram_tensor("WkT", [E, E], BF16, kind="ExternalInput").ap()
    wvT_d = nc.dram_tensor("WvT", [E, E], BF16, kind="ExternalInput").ap()
    g_d = b_d = None
    if apply_gb:
        g_d = nc.dram_tensor("ln_gamma", [E], F32, kind="ExternalInput").ap()
        b_d = nc.dram_tensor("ln_beta", [E], F32, kind="ExternalInput").ap()
    out_d = nc.dram_tensor("out", [S, E], F32, kind="ExternalOutput").ap()
    with tile.TileContext(nc) as tc:
        _emit(nc, tc, xT_d, wqT_d, wkT_d, wvT_d, g_d, b_d, out_d, apply_gb)
    nc.compile()
    return nc


_CACHE = {}


def _get_nc(apply_gb=True):
    key = ("nc", apply_gb)
    if key not in _CACHE:
        _CACHE[key] = build_attention(apply_gb)
    return _CACHE[key]


def kernel(x, Wq, Wk, Wv, ln_gamma, ln_beta):
    g = np.ascontiguousarray(ln_gamma, dtype=np.float32)
    b = np.ascontiguousarray(ln_beta, dtype=np.float32)
    apply_gb = not (np.all(g == 1.0) and np.all(b == 0.0))
    nc = _get_nc(apply_gb)
    B = x.shape[0]
    bf16 = ml_dtypes.bfloat16
    wq = np.ascontiguousarray(np.asarray(Wq, dtype=np.float32).T.astype(bf16))
    wk = np.ascontiguousarray(np.asarray(Wk, dtype=np.float32).T.astype(bf16))
    wv = np.ascontiguousarray(np.asarray(Wv, dtype=np.float32).T.astype(bf16))
    in_maps = []
    for i in range(B):
        m = {
            "xT": np.ascontiguousarray(
                np.asarray(x[i], dtype=np.float32).T.astype(bf16)),
            "WqT": wq, "WkT": wk, "WvT": wv,
        }
        if apply_gb:
            m["ln_gamma"] = g
            m["ln_beta"] = b
        in_maps.append(m)
    try:
        res = run_bass_kernel_spmd(nc, in_maps, core_ids=list(range(B)))
    except Exception:
        # transient accelerator failures (e.g. NRT_EXEC_UNIT_UNRECOVERABLE
        # after a prior run wedged the device) usually clear on retry
        import time as _time
        _time.sleep(30)
        res = run_bass_kernel_spmd(nc, in_maps, core_ids=list(range(B)))
    return np.stack([res.results[i]["out"] for i in range(B)], axis=0)


# revision 33
# speedup vs baseline: 1.0573x; 1.0573x over previous
"""Multi-head attention + LayerNorm Trainium2 kernel (v3).

Full inputs: x [8, 1024, 512], Wq/Wk/Wv [512, 512], ln_gamma/ln_beta [512].
Data-parallel over batch: one batch element per NeuronCore (8 cores), no
collectives. Host preprocessing ships transposed bf16 views of the inputs
(xT [E,S], WqT/WkT/WvT [E,E]) so the device does no layout transposes.

Per-core dataflow (S=1024, E=512, H=8 heads, D=64 head dim):
  1. Projections q^T/k^T in [e_out, s] layout (bf16 operands, f32 PSUM).
     DVE quantizes q to fp8e4m3 as a (hi, residual) pair and k to a
     duplicated fp8 pair, enabling DoubleRow QK matmuls: the pair dim
     contracts (k,k)x(q_hi,q_res) = k·(q_hi+q_res), i.e. q at ~bf16
     precision, k at fp8, 0.5 cycles/output column.
  2. scores^T [sk, sq] per (head, sk-tile); exp with the 1/sqrt(E) scale
     fused, bf16 out, split between ScalarE (activation Exp) and the
     otherwise-idle GPSIMD engine (tensor_tensor pow: e^(s*x) = b^x with
     b = e^s, measured exact to ~2e-6 on hw). No max subtraction needed:
     scores are ~N(0, 0.35).
  3. AV in [sq, d] orientation: out[sq-128, D+1] accumulates over sk
     chunks with lhsT = exp tile (free dim = sq chunk), rhs = [v | 1]
     so the softmax normalizer lands in column D as a per-partition
     scalar. Output free size is 65, so this is ~2x cheaper on the PE
     than the [d, sq] orientation and needs no output transpose.
  4. Per (h, tq): GPSIMD divides cols 0..D by col D into o_all (keeps
     the PSUM accumulator ring off the in-order DVE queue), DVE
     bn_stats; LayerNorm + DMA out at the tail.
"""

import math
import numpy as np
import ml_dtypes
from contextlib import ExitStack

import concourse.bass as bass
import concourse.tile as tile
from concourse import bacc, mybir
from concourse.bass_utils import run_bass_kernel_spmd

S = 1024
E = 512
H = 8
D = 64
P = 128
NE = E // P   # 4 e-chunks
NS = S // P   # 8 s-tiles
DP1 = D + 1   # head dim + normalizer column
PH = DP1      # per-head stride in vext
SCALE = float(E) ** -0.5
BASE = math.exp(SCALE)
EPS = 1e-5

F32 = mybir.dt.float32
F32R = mybir.dt.float32r
BF16 = mybir.dt.bfloat16
FP8 = mybir.dt.float8e4
AF = mybir.ActivationFunctionType
ALU = mybir.AluOpType
DR = mybir.MatmulPerfMode.DoubleRow

QK_FP8 = True   # DoubleRow fp8 QK (k fp8, q hi+res fp8 pair)
K_BCAST = True  # stride-0 broadcast of k in the DoubleRow pair (no dup copy)
# exp tiles relayed (DVE copy to SBUF, then GPSIMD pow) off ScalarE:
# (head, sk-tile) pairs.  GPSIMD cannot read PSUM, hence the relay.
EXP_POOL = {(h, tk) for h in range(2, H) for tk in (2, 5)}


def _emit(nc, tc, xT_d, wqT_d, wkT_d, wvT_d, g_d, b_d, out_d, apply_gb):
    ctx = ExitStack()
    with ctx:
        persist = ctx.enter_context(tc.tile_pool(name="persist", bufs=1))
        ps_pool = ctx.enter_context(tc.tile_pool(name="ps", bufs=2, space="PSUM"))
        expp = ctx.enter_context(tc.tile_pool(name="expp", bufs=16))
        finp = ctx.enter_context(tc.tile_pool(name="fin", bufs=4))

        eps_t = persist.tile([P, 1], F32, tag="eps", name="eps")
        nc.vector.memset(eps_t, EPS)
        base_t = persist.tile([P, 1], F32, tag="base", name="base")
        nc.vector.memset(base_t, BASE)
        scr = persist.tile([P, 1], F32, tag="scr", name="scr")
        if apply_gb:
            gam_b = persist.tile([P, E], F32, tag="gam", name="gam")
            nc.gpsimd.dma_start(out=gam_b, in_=g_d.partition_broadcast(P))
            bet_b = persist.tile([P, E], F32, tag="bet", name="bet")
            nc.gpsimd.dma_start(out=bet_b, in_=b_d.partition_broadcast(P))

        xT = persist.tile([P, NE, S], BF16, tag="xT", name="xT")
        wT = persist.tile([P, 3, NE, E], BF16, tag="wT", name="wT")
        if QK_FP8:
            q8 = persist.tile([P, NE, 2, S], FP8, tag="q8", name="q8")
            k8 = persist.tile([P, NE, 2, S], FP8, tag="k8", name="k8")
        else:
            qT = persist.tile([P, NE, S], BF16, tag="q8", name="qT")
            kT = persist.tile([P, NE, S], BF16, tag="k8", name="kT")
        vext = persist.tile([P, NS, H * PH], BF16, tag="vext", name="vext")
        o_all = persist.tile([P, NS, E], F32, tag="o_all", name="o_all")
        st_all = persist.tile([P, NS, H, 6], F32, tag="st_all", name="st_all")

        # loads, priority-ordered for the first QK tiles: x half 0, then the
        # chunk-0 column slices of Wq/Wk (all the first head pair needs),
        # x half 1, the Wq/Wk remainders, Wv.
        def dma_x_half(n):
            nc.sync.dma_start(
                out=xT[:, :, n * 512:(n + 1) * 512],
                in_=xT_d.rearrange("(c p) s -> p c s", p=P)[:, :, n * 512:(n + 1) * 512],
            )

        def dma_w(wi, w_d, lo, hi):
            nc.sync.dma_start(
                out=wT[:, wi, :, lo:hi],
                in_=w_d.rearrange("(c p) e -> p c e", p=P)[:, :, lo:hi],
            )

        dma_x_half(0)
        dma_w(0, wqT_d, 0, P)
        dma_w(1, wkT_d, 0, P)
        dma_x_half(1)
        dma_w(0, wqT_d, P, E)
        dma_w(1, wkT_d, P, E)
        dma_w(2, wvT_d, 0, E)

        def proj_half(wi, c, n):
            """q/k chunk c, s-half n: psum [P, 512] so its preps only wait
            on this half's accumulation group.  Own 1-bank tag so projection
            tiles never block the score-tile ring (which would starve the
            exp stream)."""
            pp = ps_pool.tile([P, 512], F32, tag="pp", bufs=1,
                              name=f"pp{wi}_{c}_{n}")
            for ce in range(NE):
                nc.tensor.matmul(
                    out=pp,
                    lhsT=wT[:, wi, ce, c * P:(c + 1) * P],
                    rhs=xT[:, ce, n * 512:(n + 1) * 512],
                    start=(ce == 0), stop=(ce == NE - 1),
                )
            return pp

        def prep_q(c, n, pp):
            sl = slice(n * 512, (n + 1) * 512)
            if QK_FP8:
                nc.vector.tensor_copy(out=q8[:, c, 0, sl], in_=pp)
                nc.vector.tensor_tensor(
                    out=q8[:, c, 1, sl], in0=pp, in1=q8[:, c, 0, sl],
                    op=ALU.subtract,
                )
            else:
                nc.vector.tensor_copy(out=qT[:, c, sl], in_=pp)

        def prep_k(c, n, pp):
            sl = slice(n * 512, (n + 1) * 512)
            if QK_FP8:
                nc.vector.tensor_copy(out=k8[:, c, 0, sl], in_=pp)
                if not K_BCAST:
                    nc.gpsimd.tensor_copy(out=k8[:, c, 1, sl],
                                          in_=k8[:, c, 0, sl])
            else:
                nc.vector.tensor_copy(out=kT[:, c, sl], in_=pp)

        def prep_qk_chunk(c):
            for n in range(2):
                prep_q(c, n, proj_half(0, c, n))
                prep_k(c, n, proj_half(1, c, n))

        def vproj(t_i):
            pv = ps_pool.tile([P, E], F32, tag="pp", bufs=1, name=f"pv{t_i}")
            for ce in range(NE):
                nc.tensor.matmul(
                    out=pv,
                    lhsT=xT[:, ce, t_i * P:(t_i + 1) * P],
                    rhs=wT[:, 2, ce, :],
                    start=(ce == 0), stop=(ce == NE - 1),
                )
            vdst = vext[:, t_i, :].rearrange("p (h c) -> p h c", c=PH)[:, :, 0:D]
            nc.vector.tensor_copy(out=vdst, in_=pv.rearrange("p (h d) -> p h d", d=D))

        exp_tiles = {}

        def qk_mm(sp, h, tk, n):
            c, b = h // 2, D * (h % 2)
            if QK_FP8:
                if K_BCAST:
                    lhsT = k8[b:b + D, c, 0:1, tk * P:(tk + 1) * P] \
                        .broadcast_to([D, 2, P])
                else:
                    lhsT = k8[b:b + D, c, :, tk * P:(tk + 1) * P]
                nc.tensor.matmul(
                    out=sp[:, n * 512:(n + 1) * 512],
                    lhsT=lhsT,
                    rhs=q8[b:b + D, c, :, n * 512:(n + 1) * 512],
                    start=True, stop=True,
                    perf_mode=DR,
                )
            else:
                nc.tensor.matmul(
                    out=sp[:, n * 512:(n + 1) * 512],
                    lhsT=kT[b:b + D, c, tk * P:(tk + 1) * P],
                    rhs=qT[b:b + D, c, n * 512:(n + 1) * 512],
                    start=True, stop=True,
                )

        def exp_dst(h, tk):
            key = (h, tk // 2)
            if key not in exp_tiles:
                exp_tiles[key] = expp.tile([P, 2, S], BF16, tag="exp",
                                           name=f"e{h}_{tk}")
            return exp_tiles[key][:, tk % 2, :]

        def exp_half(sp, h, tk, n):
            sl = slice(n * 512, (n + 1) * 512)
            nc.scalar.activation(
                out=exp_dst(h, tk)[:, sl], in_=sp[:, sl], func=AF.Exp,
                scale=SCALE,
            )

        def qk(h, tk):
            """scores^T tile [sk=128, sq=1024] for head h, sk tile tk + exp."""
            sp = ps_pool.tile([P, S], F32, tag="ps", name=f"sc{h}_{tk}")
            qk_mm(sp, h, tk, 0)
            qk_mm(sp, h, tk, 1)
            dst = exp_dst(h, tk)
            if (h, tk) in EXP_POOL:
                stage = expp.tile([P, S], F32, tag="stage", bufs=3,
                                  name=f"st{h}_{tk}")
                for n in range(2):
                    sl = slice(n * 512, (n + 1) * 512)
                    nc.vector.tensor_copy(out=stage[:, sl], in_=sp[:, sl])
                nc.gpsimd.tensor_tensor(
                    out=dst, in0=base_t.broadcast_to([P, S]), in1=stage,
                    op=ALU.pow,
                )
            else:
                nc.scalar.activation(out=dst, in_=sp, func=AF.Exp, scale=SCALE)

        def av_head(h, tq, pu):
            """U[sq-tile tq, D+1] for head h: accumulate over all sk chunks."""
            for tk in range(NS):
                nc.tensor.matmul(
                    out=pu,
                    lhsT=exp_tiles[(h, tk // 2)][:, tk % 2, tq * P:(tq + 1) * P],
                    rhs=vext[:, tk, h * PH:h * PH + DP1],
                    start=(tk == 0), stop=(tk == NS - 1),
                )

        def fin_head(h, tq, pu, on_act=False):
            rc = finp.tile([P, 1], F32, tag="rc", name=f"rc{h}_{tq}")
            nc.vector.reciprocal(out=rc, in_=pu[:, D:DP1])
            if on_act:
                nc.scalar.activation(
                    out=o_all[:, tq, h * D:(h + 1) * D],
                    in_=pu[:, 0:D], func=AF.Copy, scale=rc,
                )
            else:
                nc.vector.tensor_scalar_mul(
                    out=o_all[:, tq, h * D:(h + 1) * D],
                    in0=pu[:, 0:D], scalar1=rc,
                )
            nc.vector.bn_stats(
                out=st_all[:, tq, h, :],
                in_=o_all[:, tq, h * D:(h + 1) * D],
            )

        def layer_norm(tq, xc_eng=None):
            mv = finp.tile([P, 2], F32, tag="mv", name=f"mv{tq}")
            nc.vector.bn_aggr(out=mv, in_=st_all[:, tq, :, :])
            sd = finp.tile([P, 1], F32, tag="sd", name=f"sd{tq}")
            nc.scalar.activation(out=sd, in_=mv[:, 1:2], func=AF.Sqrt, bias=eps_t)
            rs = finp.tile([P, 1], F32, tag="rs", name=f"rs{tq}")
            nc.vector.reciprocal(out=rs, in_=sd)
            xc = finp.tile([P, E], F32, tag="xc", name=f"xc{tq}")
            eng = xc_eng or nc.gpsimd
            eng.tensor_scalar(
                out=xc, in0=o_all[:, tq, :],
                scalar1=mv[:, 0:1], scalar2=rs,
                op0=ALU.subtract, op1=ALU.mult,
            )
            if apply_gb:
                eng.tensor_mul(out=xc, in0=xc, in1=gam_b)
                eng.tensor_add(out=xc, in0=xc, in1=bet_b)
            nc.sync.dma_start(out=out_d[tq * P:(tq + 1) * P, :], in_=xc)

        # AV accumulators: PSUM hazards are tracked per TILE, so every live
        # accumulator must be its own pool tile (each is bank-rounded
        # anyway).  Ring of 3 one-bank tiles under tag "u".
        pu_ctr = [0]

        def pu_slot():
            pu_ctr[0] += 1
            return ps_pool.tile([P, DP1], F32, tag="u", bufs=3,
                                name=f"pu{pu_ctr[0]}")[:, 0:DP1]

        # ---- warmup: junk matmuls ramp the PE out of its low p-state while
        # the first DMAs are still in flight (cycles the "u" ring, which
        # sees real accumulations only much later)
        wu = persist.tile([P, 640], BF16, tag="wu", name="wu")
        nc.gpsimd.memset(wu, 0.0)
        for i in range(40):
            wup = ps_pool.tile([P, DP1], F32, tag="u", bufs=3,
                               name=f"wup{i}")
            nc.tensor.matmul(out=wup, lhsT=wu[:, 0:P], rhs=wu[:, P:P + DP1],
                             start=True, stop=True)

        for t_i in range(NS):
            ones_v = vext[:, t_i, :].rearrange("p (h c) -> p h c", c=PH)[:, :, D:DP1]
            nc.gpsimd.memset(ones_v, 1.0)

        # ---- stage 1: q/k chunk 0; the first two score tiles run their
        # s-half 0 as soon as the half-0 preps land, with half-1 following.
        # v projection + the other q/k chunks ride the ScalarE-paced slack
        # one projection-half at a time so the score-tile ring never blocks.
        prep_q(0, 0, proj_half(0, 0, 0))
        prep_k(0, 0, proj_half(1, 0, 0))
        sp0 = ps_pool.tile([P, S], F32, tag="ps", name="sc0_0")
        qk_mm(sp0, 0, 0, 0)
        exp_half(sp0, 0, 0, 0)
        sp1 = ps_pool.tile([P, S], F32, tag="ps", name="sc0_1")
        qk_mm(sp1, 0, 1, 0)
        exp_half(sp1, 0, 1, 0)
        prep_q(0, 1, proj_half(0, 0, 1))
        prep_k(0, 1, proj_half(1, 0, 1))
        qk_mm(sp0, 0, 0, 1)
        exp_half(sp0, 0, 0, 1)
        qk_mm(sp1, 0, 1, 1)
        exp_half(sp1, 0, 1, 1)

        fillers = [lambda t_i=t_i: vproj(t_i) for t_i in range(NS)]
        for c in (1, 2, 3):
            for n in range(2):
                fillers.append(lambda c=c, n=n: prep_q(c, n, proj_half(0, c, n)))
                fillers.append(lambda c=c, n=n: prep_k(c, n, proj_half(1, c, n)))

        def filler():
            if fillers:
                fillers.pop(0)()

        for tk in range(2, NS):
            qk(0, tk)
            filler()
        for tk in range(NS):
            qk(1, tk)
            filler()

        # ---- stage 2: pair loops: AV of pair p, QK of pair p+1 ----------
        for p in range(H // 2 - 1):
            h0, h1 = 2 * p, 2 * p + 1
            for tq in range(NS):
                qk(2 * p + 2, tq)
                filler()
                qk(2 * p + 3, tq)
                pu0 = pu_slot()
                av_head(h0, tq, pu0)
                fin_head(h0, tq, pu0)
                pu1 = pu_slot()
                av_head(h1, tq, pu1)
                fin_head(h1, tq, pu1)

        # pre-switch the ACT table to the sqrt set now that the last exp has
        # been emitted, so the switch overlaps the final AV instead of the tail
        nc.scalar.activation(out=scr, in_=eps_t, func=AF.Sqrt)

        # last pair: no next-pair QK to interleave.  The score-tile and
        # projection rings are free now, so the accumulator ring widens to
        # ~6 distinct tiles — the AV stream runs nearly back-to-back on the
        # PE while fin / LayerNorm chains drain behind it on the other
        # engines.
        def last_slot(i):
            kind = i % 6
            if kind == 1:
                return ps_pool.tile([P, S], F32, tag="ps",
                                    name=f"fps{i}")[:, 0:DP1]
            if kind == 4:
                return ps_pool.tile([P, 512], F32, tag="pp", bufs=1,
                                    name=f"fpp{i}")[:, 0:DP1]
            return pu_slot()

        p = H // 2 - 1
        for tq in range(NS):
            pus = {}
            for h in (2 * p, 2 * p + 1):
                pus[h] = last_slot(2 * tq + (h % 2))
                av_head(h, tq, pus[h])
            for h in (2 * p, 2 * p + 1):
                fin_head(h, tq, pus[h], on_act=(h % 2 == 0))
            layer_norm(tq, xc_eng=(nc.vector if tq % 2 else nc.gpsimd))


def build_attention(apply_gb=True):
    nc = bacc.Bacc("TRN2", target_bir_lowering=False, debug=False)
    xT_d = nc.dram_tensor("xT", [E, S], BF16, kind="ExternalInput").ap()
    wqT_d = nc.dram_tensor("WqT", [E, E], BF16, kind="ExternalInput").ap()
    wkT_d = nc.dram_tensor("WkT", [E, E], BF16, kind="ExternalInput").ap()
    wvT_d = nc.dram_tensor("WvT", [E, E], BF16, kind="ExternalInput").ap()
    g_d = b_d = None
    if apply_gb:
        g_d = nc.dram_tensor("ln_gamma", [E], F32, kind="ExternalInput").ap()
        b_d = nc.dram_tensor("ln_beta", [E], F32, kind="ExternalInput").ap()
    out_d = nc.dram_tensor("out", [S, E], F32, kind="ExternalOutput").ap()
    with tile.TileContext(nc) as tc:
        _emit(nc, tc, xT_d, wqT_d, wkT_d, wvT_d, g_d, b_d, out_d, apply_gb)
    nc.compile()
    return nc


_CACHE = {}


def _get_nc(apply_gb=True):
    key = ("nc", apply_gb)
    if key not in _CACHE:
        _CACHE[key] = build_attention(apply_gb)
    return _CACHE[key]


def kernel(x, Wq, Wk, Wv, ln_gamma, ln_beta):
    g = np.ascontiguousarray(ln_gamma, dtype=np.float32)
    b = np.ascontiguousarray(ln_beta, dtype=np.float32)
    apply_gb = not (np.all(g == 1.0) and np.all(b == 0.0))
    nc = _get_nc(apply_gb)
    B = x.shape[0]
    bf16 = ml_dtypes.bfloat16
    wq = np.ascontiguousarray(np.asarray(Wq, dtype=np.float32).T.astype(bf16))
    wk = np.ascontiguousarray(np.asarray(Wk, dtype=np.float32).T.astype(bf16))
    wv = np.ascontiguousarray(np.asarray(Wv, dtype=np.float32).T.astype(bf16))
    in_maps = []
    for i in range(B):
        m = {
            "xT": np.ascontiguousarray(
                np.asarray(x[i], dtype=np.float32).T.astype(bf16)),
            "WqT": wq, "WkT": wk, "WvT": wv,
        }
        if apply_gb:
            m["ln_gamma"] = g
            m["ln_beta"] = b
        in_maps.append(m)
    try:
        res = run_bass_kernel_spmd(nc, in_maps, core_ids=list(range(B)))
    except Exception:
        # transient accelerator failures (e.g. NRT_EXEC_UNIT_UNRECOVERABLE
        # after a prior run wedged the device) usually clear on retry
        import time as _time
        _time.sleep(30)
        res = run_bass_kernel_spmd(nc, in_maps, core_ids=list(range(B)))
    return np.stack([res.results[i]["out"] for i in range(B)], axis=0)


# revision 49
# speedup vs baseline: 1.0596x; 1.0021x over previous
"""Multi-head attention + LayerNorm Trainium2 kernel (v3).

Full inputs: x [8, 1024, 512], Wq/Wk/Wv [512, 512], ln_gamma/ln_beta [512].
Data-parallel over batch: one batch element per NeuronCore (8 cores), no
collectives. Host preprocessing ships transposed bf16 views of the inputs
(xT [E,S], WqT/WkT/WvT [E,E]) so the device does no layout transposes.

Per-core dataflow (S=1024, E=512, H=8 heads, D=64 head dim):
  1. Projections q^T/k^T in [e_out, s] layout (bf16 operands, f32 PSUM).
     DVE quantizes q to fp8e4m3 as a (hi, residual) pair and k to a
     duplicated fp8 pair, enabling DoubleRow QK matmuls: the pair dim
     contracts (k,k)x(q_hi,q_res) = k·(q_hi+q_res), i.e. q at ~bf16
     precision, k at fp8, 0.5 cycles/output column.
  2. scores^T [sk, sq] per (head, sk-tile); exp with the 1/sqrt(E) scale
     fused, bf16 out, split between ScalarE (activation Exp) and the
     otherwise-idle GPSIMD engine (tensor_tensor pow: e^(s*x) = b^x with
     b = e^s, measured exact to ~2e-6 on hw). No max subtraction needed:
     scores are ~N(0, 0.35).
  3. AV in [sq, d] orientation: out[sq-128, D+1] accumulates over sk
     chunks with lhsT = exp tile (free dim = sq chunk), rhs = [v | 1]
     so the softmax normalizer lands in column D as a per-partition
     scalar. Output free size is 65, so this is ~2x cheaper on the PE
     than the [d, sq] orientation and needs no output transpose.
  4. Per (h, tq): GPSIMD divides cols 0..D by col D into o_all (keeps
     the PSUM accumulator ring off the in-order DVE queue), DVE
     bn_stats; LayerNorm + DMA out at the tail.
"""

import math
import numpy as np
import ml_dtypes
from contextlib import ExitStack

import concourse.bass as bass
import concourse.tile as tile
from concourse import bacc, mybir
from concourse.bass_utils import run_bass_kernel_spmd

S = 1024
E = 512
H = 8
D = 64
P = 128
NE = E // P   # 4 e-chunks
NS = S // P   # 8 s-tiles
DP1 = D + 1   # head dim + normalizer column
PH = DP1      # per-head stride in vext
SCALE = float(E) ** -0.5
BASE = math.exp(SCALE)
EPS = 1e-5

F32 = mybir.dt.float32
F32R = mybir.dt.float32r
BF16 = mybir.dt.bfloat16
FP8 = mybir.dt.float8e4
AF = mybir.ActivationFunctionType
ALU = mybir.AluOpType
DR = mybir.MatmulPerfMode.DoubleRow

QK_FP8 = True   # DoubleRow fp8 QK (k fp8, q hi+res fp8 pair)
K_BCAST = True  # stride-0 broadcast of k in the DoubleRow pair (no dup copy)
# exp tiles relayed (DVE copy to SBUF, then GPSIMD pow) off ScalarE:
# (head, sk-tile) pairs.  GPSIMD cannot read PSUM, hence the relay.
import os
EXP_POOL = set() if os.environ.get('NO_RELAY') else \
    {(h, tk) for h in range(2, H) for tk in (2, 5)}
RELAY_DMA = False  # relay scores PSUM->SBUF via DMA engines (idle mid-kernel)


def _emit(nc, tc, xT_d, wqT_d, wkT_d, wvT_d, g_d, b_d, out_d, apply_gb):
    ctx = ExitStack()
    with ctx:
        persist = ctx.enter_context(tc.tile_pool(name="persist", bufs=1))
        ps_pool = ctx.enter_context(tc.tile_pool(name="ps", bufs=2, space="PSUM"))
        expp = ctx.enter_context(tc.tile_pool(name="expp", bufs=16))
        finp = ctx.enter_context(tc.tile_pool(name="fin", bufs=4))

        eps_t = persist.tile([P, 1], F32, tag="eps", name="eps")
        nc.vector.memset(eps_t, EPS)
        base_t = persist.tile([P, 1], F32, tag="base", name="base")
        nc.vector.memset(base_t, BASE)
        scr = persist.tile([P, 1], F32, tag="scr", name="scr")
        if apply_gb:
            gam_b = persist.tile([P, E], F32, tag="gam", name="gam")
            nc.gpsimd.dma_start(out=gam_b, in_=g_d.partition_broadcast(P))
            bet_b = persist.tile([P, E], F32, tag="bet", name="bet")
            nc.gpsimd.dma_start(out=bet_b, in_=b_d.partition_broadcast(P))

        xT = persist.tile([P, NE, S], BF16, tag="xT", name="xT")
        wT = persist.tile([P, 3, NE, E], BF16, tag="wT", name="wT")
        if QK_FP8:
            q8 = persist.tile([P, NE, 2, S], FP8, tag="q8", name="q8")
            k8 = persist.tile([P, NE, 2, S], FP8, tag="k8", name="k8")
        else:
            qT = persist.tile([P, NE, S], BF16, tag="q8", name="qT")
            kT = persist.tile([P, NE, S], BF16, tag="k8", name="kT")
        vext = persist.tile([P, NS, H * PH], BF16, tag="vext", name="vext")
        o_all = persist.tile([P, NS, E], F32, tag="o_all", name="o_all")
        st_all = persist.tile([P, NS, H, 6], F32, tag="st_all", name="st_all")

        # loads, priority-ordered for the first QK tiles: x half 0 (per
        # chunk, so projection accumulation starts while later chunks are
        # in flight), the chunk-0 column slices of Wk/Wq, x half 1, the
        # remainders, Wv.
        def dma_x(c, n):
            nc.sync.dma_start(
                out=xT[:, c, n * 512:(n + 1) * 512],
                in_=xT_d[c * P:(c + 1) * P, n * 512:(n + 1) * 512],
            )

        def dma_w(wi, w_d, lo, hi):
            nc.sync.dma_start(
                out=wT[:, wi, :, lo:hi],
                in_=w_d.rearrange("(c p) e -> p c e", p=P)[:, :, lo:hi],
            )

        dma_x(0, 0)
        dma_w(1, wkT_d, 0, P)
        dma_w(0, wqT_d, 0, P)
        for c in range(1, NE):
            dma_x(c, 0)
        for c in range(NE):
            dma_x(c, 1)
        dma_w(1, wkT_d, P, E)
        dma_w(0, wqT_d, P, E)
        dma_w(2, wvT_d, 0, E)

        def proj_half(wi, c, n):
            """q/k chunk c, s-half n: psum [P, 512] so its preps only wait
            on this half's accumulation group.  Own 1-bank tag so projection
            tiles never block the score-tile ring (which would starve the
            exp stream)."""
            pp = ps_pool.tile([P, 512], F32, tag="pp", bufs=1,
                              name=f"pp{wi}_{c}_{n}")
            for ce in range(NE):
                nc.tensor.matmul(
                    out=pp,
                    lhsT=wT[:, wi, ce, c * P:(c + 1) * P],
                    rhs=xT[:, ce, n * 512:(n + 1) * 512],
                    start=(ce == 0), stop=(ce == NE - 1),
                )
            return pp

        def prep_q(c, n, pp):
            sl = slice(n * 512, (n + 1) * 512)
            if QK_FP8:
                nc.vector.tensor_copy(out=q8[:, c, 0, sl], in_=pp)
                nc.vector.tensor_tensor(
                    out=q8[:, c, 1, sl], in0=pp, in1=q8[:, c, 0, sl],
                    op=ALU.subtract,
                )
            else:
                nc.vector.tensor_copy(out=qT[:, c, sl], in_=pp)

        def prep_k(c, n, pp):
            sl = slice(n * 512, (n + 1) * 512)
            if QK_FP8:
                nc.vector.tensor_copy(out=k8[:, c, 0, sl], in_=pp)
                if not K_BCAST:
                    nc.gpsimd.tensor_copy(out=k8[:, c, 1, sl],
                                          in_=k8[:, c, 0, sl])
            else:
                nc.vector.tensor_copy(out=kT[:, c, sl], in_=pp)

        def prep_qk_chunk(c):
            for n in range(2):
                prep_k(c, n, proj_half(1, c, n))
                prep_q(c, n, proj_half(0, c, n))

        def vproj(t_i):
            pv = ps_pool.tile([P, E], F32, tag="pp", bufs=1, name=f"pv{t_i}")
            for ce in range(NE):
                nc.tensor.matmul(
                    out=pv,
                    lhsT=xT[:, ce, t_i * P:(t_i + 1) * P],
                    rhs=wT[:, 2, ce, :],
                    start=(ce == 0), stop=(ce == NE - 1),
                )
            vdst = vext[:, t_i, :].rearrange("p (h c) -> p h c", c=PH)[:, :, 0:D]
            nc.vector.tensor_copy(out=vdst, in_=pv.rearrange("p (h d) -> p h d", d=D))

        exp_tiles = {}

        def qk_mm(sp, h, tk, n):
            c, b = h // 2, D * (h % 2)
            if QK_FP8:
                if K_BCAST:
                    lhsT = k8[b:b + D, c, 0:1, tk * P:(tk + 1) * P] \
                        .broadcast_to([D, 2, P])
                else:
                    lhsT = k8[b:b + D, c, :, tk * P:(tk + 1) * P]
                nc.tensor.matmul(
                    out=sp[:, n * 512:(n + 1) * 512],
                    lhsT=lhsT,
                    rhs=q8[b:b + D, c, :, n * 512:(n + 1) * 512],
                    start=True, stop=True,
                    perf_mode=DR,
                )
            else:
                nc.tensor.matmul(
                    out=sp[:, n * 512:(n + 1) * 512],
                    lhsT=kT[b:b + D, c, tk * P:(tk + 1) * P],
                    rhs=qT[b:b + D, c, n * 512:(n + 1) * 512],
                    start=True, stop=True,
                )

        def exp_dst(h, tk):
            key = (h, tk // 2)
            if key not in exp_tiles:
                exp_tiles[key] = expp.tile([P, 2, S], BF16, tag="exp",
                                           name=f"e{h}_{tk}")
            return exp_tiles[key][:, tk % 2, :]

        def exp_half(sp, h, tk, n):
            sl = slice(n * 512, (n + 1) * 512)
            nc.scalar.activation(
                out=exp_dst(h, tk)[:, sl], in_=sp[:, sl], func=AF.Exp,
                scale=SCALE,
            )

        def qk(h, tk):
            """scores^T tile [sk=128, sq=1024] for head h, sk tile tk + exp."""
            sp = ps_pool.tile([P, S], F32, tag="ps", name=f"sc{h}_{tk}")
            qk_mm(sp, h, tk, 0)
            qk_mm(sp, h, tk, 1)
            dst = exp_dst(h, tk)
            if (h, tk) in EXP_POOL:
                stage = expp.tile([P, S], F32, tag="stage", bufs=3,
                                  name=f"st{h}_{tk}")
                if RELAY_DMA:
                    nc.sync.dma_start(out=stage, in_=sp)
                else:
                    for n in range(2):
                        sl = slice(n * 512, (n + 1) * 512)
                        nc.vector.tensor_copy(out=stage[:, sl], in_=sp[:, sl])
                nc.gpsimd.tensor_tensor(
                    out=dst, in0=base_t.broadcast_to([P, S]), in1=stage,
                    op=ALU.pow,
                )
            else:
                nc.scalar.activation(out=dst, in_=sp, func=AF.Exp, scale=SCALE)

        def av_head(h, tq, pu):
            """U[sq-tile tq, D+1] for head h: accumulate over all sk chunks."""
            for tk in range(NS):
                nc.tensor.matmul(
                    out=pu,
                    lhsT=exp_tiles[(h, tk // 2)][:, tk % 2, tq * P:(tq + 1) * P],
                    rhs=vext[:, tk, h * PH:h * PH + DP1],
                    start=(tk == 0), stop=(tk == NS - 1),
                )

        def fin_head(h, tq, pu, on_act=False):
            rc = finp.tile([P, 1], F32, tag="rc", name=f"rc{h}_{tq}")
            nc.vector.reciprocal(out=rc, in_=pu[:, D:DP1])
            if on_act:
                nc.scalar.activation(
                    out=o_all[:, tq, h * D:(h + 1) * D],
                    in_=pu[:, 0:D], func=AF.Copy, scale=rc,
                )
            else:
                nc.vector.tensor_scalar_mul(
                    out=o_all[:, tq, h * D:(h + 1) * D],
                    in0=pu[:, 0:D], scalar1=rc,
                )
            nc.vector.bn_stats(
                out=st_all[:, tq, h, :],
                in_=o_all[:, tq, h * D:(h + 1) * D],
            )

        def layer_norm(tq, xc_eng=None, nst=H):
            mv = finp.tile([P, 2], F32, tag="mv", name=f"mv{tq}")
            nc.vector.bn_aggr(out=mv, in_=st_all[:, tq, 0:nst, :])
            sd = finp.tile([P, 1], F32, tag="sd", name=f"sd{tq}")
            nc.scalar.activation(out=sd, in_=mv[:, 1:2], func=AF.Sqrt, bias=eps_t)
            rs = finp.tile([P, 1], F32, tag="rs", name=f"rs{tq}")
            nc.vector.reciprocal(out=rs, in_=sd)
            xc = finp.tile([P, E], F32, tag="xc", bufs=8, name=f"xc{tq}")
            eng = xc_eng or nc.gpsimd
            eng.tensor_scalar(
                out=xc, in0=o_all[:, tq, :],
                scalar1=mv[:, 0:1], scalar2=rs,
                op0=ALU.subtract, op1=ALU.mult,
            )
            if apply_gb:
                eng.tensor_mul(out=xc, in0=xc, in1=gam_b)
                eng.tensor_add(out=xc, in0=xc, in1=bet_b)
            nc.sync.dma_start(out=out_d[tq * P:(tq + 1) * P, :], in_=xc)

        # AV accumulators: PSUM hazards are tracked per TILE, so every live
        # accumulator must be its own pool tile (each is bank-rounded
        # anyway).  Ring of 3 one-bank tiles under tag "u".
        pu_ctr = [0]

        def pu_slot():
            pu_ctr[0] += 1
            return ps_pool.tile([P, DP1], F32, tag="u", bufs=3,
                                name=f"pu{pu_ctr[0]}")[:, 0:DP1]

        # ---- warmup: junk matmuls ramp the PE out of its low p-state while
        # the first DMAs are still in flight (cycles the "u" ring, which
        # sees real accumulations only much later)
        wu = persist.tile([P, 640], BF16, tag="wu", name="wu")
        nc.gpsimd.memset(wu, 0.0)
        for i in range(120):
            wup = ps_pool.tile([P, DP1], F32, tag="u", bufs=3,
                               name=f"wup{i}")
            nc.tensor.matmul(out=wup, lhsT=wu[:, 0:P], rhs=wu[:, P:P + DP1],
                             start=True, stop=True)

        for t_i in range(NS):
            ones_v = vext[:, t_i, :].rearrange("p (h c) -> p h c", c=PH)[:, :, D:DP1]
            nc.gpsimd.memset(ones_v, 1.0)

        # ---- stage 1: q/k chunk 0; the first two score tiles run their
        # s-half 0 as soon as the half-0 preps land, with half-1 following.
        # v projection + the other q/k chunks ride the ScalarE-paced slack
        # one projection-half at a time so the score-tile ring never blocks.
        # head 0 runs as 16 half-width score tiles in s-half-major order, so
        # the exp stream starts on half 0 the moment the half-0 preps land
        # and never waits for the x-half-1 DMA / prep chain.
        prep_k(0, 0, proj_half(1, 0, 0))
        prep_q(0, 0, proj_half(0, 0, 0))

        def qk_h0_half(tk, n):
            sph = ps_pool.tile([P, 512], F32, tag="ps", name=f"h0_{tk}_{n}")
            c, b = 0, 0
            lhsT = k8[b:b + D, c, 0:1, tk * P:(tk + 1) * P] \
                .broadcast_to([D, 2, P]) if (QK_FP8 and K_BCAST) else None
            if QK_FP8:
                nc.tensor.matmul(
                    out=sph,
                    lhsT=lhsT if K_BCAST else k8[b:b + D, c, :, tk * P:(tk + 1) * P],
                    rhs=q8[b:b + D, c, :, n * 512:(n + 1) * 512],
                    start=True, stop=True, perf_mode=DR,
                )
            else:
                nc.tensor.matmul(
                    out=sph,
                    lhsT=kT[b:b + D, c, tk * P:(tk + 1) * P],
                    rhs=qT[b:b + D, c, n * 512:(n + 1) * 512],
                    start=True, stop=True,
                )
            sl = slice(n * 512, (n + 1) * 512)
            nc.scalar.activation(
                out=exp_dst(0, tk)[:, sl], in_=sph, func=AF.Exp, scale=SCALE,
            )

        for tk in range(NS):
            qk_h0_half(tk, 0)
        prep_k(0, 1, proj_half(1, 0, 1))
        prep_q(0, 1, proj_half(0, 0, 1))
        for tk in range(NS):
            qk_h0_half(tk, 1)

        # front-load the chunk-1 preps (needed by head 2/3 QK), then the v
        # projection, then chunks 2-3; stage 1 has the most DVE slack.
        fillers = []
        for n in range(2):
            fillers.append(lambda n=n: prep_k(1, n, proj_half(1, 1, n)))
            fillers.append(lambda n=n: prep_q(1, n, proj_half(0, 1, n)))
        fillers += [lambda t_i=t_i: vproj(t_i) for t_i in range(NS)]
        for c in (2, 3):
            for n in range(2):
                fillers.append(lambda c=c, n=n: prep_k(c, n, proj_half(1, c, n)))
                fillers.append(lambda c=c, n=n: prep_q(c, n, proj_half(0, c, n)))

        def filler():
            if fillers:
                fillers.pop(0)()

        for tk in range(NS):
            qk(1, tk)
            filler()
            if tk % 2 == 0:
                filler()

        # ---- stage 2: pair loops: AV of pair p, QK of pair p+1 ----------
        for p in range(H // 2 - 1):
            h0, h1 = 2 * p, 2 * p + 1
            for tq in range(NS):
                qk(2 * p + 2, tq)
                if tq % 2 == 1:
                    filler()
                qk(2 * p + 3, tq)
                pu0 = pu_slot()
                av_head(h0, tq, pu0)
                fin_head(h0, tq, pu0)
                pu1 = pu_slot()
                av_head(h1, tq, pu1)
                fin_head(h1, tq, pu1)

        # pre-switch the ACT table to the sqrt set now that the last exp has
        # been emitted, so the switch overlaps the final AV instead of the tail
        nc.scalar.activation(out=scr, in_=eps_t, func=AF.Sqrt)

        # last pair: no next-pair QK to interleave.  The score-tile and
        # projection rings are free now, so the accumulator ring widens to
        # ~6 distinct tiles — the AV stream runs nearly back-to-back on the
        # PE while fin / LayerNorm chains drain behind it on the other
        # engines.
        def last_slot(i):
            kind = i % 6
            if kind == 1:
                return ps_pool.tile([P, S], F32, tag="ps",
                                    name=f"fps{i}")[:, 0:DP1]
            if kind == 4:
                return ps_pool.tile([P, 512], F32, tag="pp", bufs=1,
                                    name=f"fpp{i}")[:, 0:DP1]
            return pu_slot()

        p = H // 2 - 1
        for tq in range(NS):
            pus = {}
            for h in (2 * p, 2 * p + 1):
                pus[h] = last_slot(2 * tq + (h % 2))
                av_head(h, tq, pus[h])
            # both heads' u/Z scaling on ScalarE (idle once exps are done);
            # one combined bn_stats over both 64-col blocks
            for h in (2 * p, 2 * p + 1):
                rc = finp.tile([P, 1], F32, tag="rc", name=f"rc{h}_{tq}")
                nc.vector.reciprocal(out=rc, in_=pus[h][:, D:DP1])
                nc.scalar.activation(
                    out=o_all[:, tq, h * D:(h + 1) * D],
                    in_=pus[h][:, 0:D], func=AF.Copy, scale=rc,
                )
            nc.vector.bn_stats(
                out=st_all[:, tq, 2 * p, :],
                in_=o_all[:, tq, 2 * p * D:(2 * p + 2) * D],
            )
            layer_norm(tq, xc_eng=(nc.vector if tq % 2 else nc.gpsimd),
                       nst=H - 1)


def build_attention(apply_gb=True):
    nc = bacc.Bacc("TRN2", target_bir_lowering=False, debug=False)
    xT_d = nc.dram_tensor("xT", [E, S], BF16, kind="ExternalInput").ap()
    wqT_d = nc.dram_tensor("WqT", [E, E], BF16, kind="ExternalInput").ap()
    wkT_d = nc.dram_tensor("WkT", [E, E], BF16, kind="ExternalInput").ap()
    wvT_d = nc.dram_tensor("WvT", [E, E], BF16, kind="ExternalInput").ap()
    g_d = b_d = None
    if apply_gb:
        g_d = nc.dram_tensor("ln_gamma", [E], F32, kind="ExternalInput").ap()
        b_d = nc.dram_tensor("ln_beta", [E], F32, kind="ExternalInput").ap()
    out_d = nc.dram_tensor("out", [S, E], F32, kind="ExternalOutput").ap()
    with tile.TileContext(nc) as tc:
        _emit(nc, tc, xT_d, wqT_d, wkT_d, wvT_d, g_d, b_d, out_d, apply_gb)
    nc.compile()
    return nc


_CACHE = {}


def _get_nc(apply_gb=True):
    key = ("nc", apply_gb)
    if key not in _CACHE:
        _CACHE[key] = build_attention(apply_gb)
    return _CACHE[key]


def kernel(x, Wq, Wk, Wv, ln_gamma, ln_beta):
    g = np.ascontiguousarray(ln_gamma, dtype=np.float32)
    b = np.ascontiguousarray(ln_beta, dtype=np.float32)
    apply_gb = not (np.all(g == 1.0) and np.all(b == 0.0))
    nc = _get_nc(apply_gb)
    B = x.shape[0]
    bf16 = ml_dtypes.bfloat16
    wq = np.ascontiguousarray(np.asarray(Wq, dtype=np.float32).T.astype(bf16))
    wk = np.ascontiguousarray(np.asarray(Wk, dtype=np.float32).T.astype(bf16))
    wv = np.ascontiguousarray(np.asarray(Wv, dtype=np.float32).T.astype(bf16))
    in_maps = []
    for i in range(B):
        m = {
            "xT": np.ascontiguousarray(
                np.asarray(x[i], dtype=np.float32).T.astype(bf16)),
            "WqT": wq, "WkT": wk, "WvT": wv,
        }
        if apply_gb:
            m["ln_gamma"] = g
            m["ln_beta"] = b
        in_maps.append(m)
    try:
        res = run_bass_kernel_spmd(nc, in_maps, core_ids=list(range(B)))
    except Exception:
        # transient accelerator failures (e.g. NRT_EXEC_UNIT_UNRECOVERABLE
        # after a prior run wedged the device) usually clear on retry
        import time as _time
        _time.sleep(30)
        res = run_bass_kernel_spmd(nc, in_maps, core_ids=list(range(B)))
    return np.stack([res.results[i]["out"] for i in range(B)], axis=0)


# revision 59
# speedup vs baseline: 1.1606x; 1.0953x over previous
"""Multi-head attention + LayerNorm Trainium2 kernel (v3).

Full inputs: x [8, 1024, 512], Wq/Wk/Wv [512, 512], ln_gamma/ln_beta [512].
Data-parallel over batch: one batch element per NeuronCore (8 cores), no
collectives. Host preprocessing ships transposed bf16 views of the inputs
(xT [E,S], WqT/WkT/WvT [E,E]) so the device does no layout transposes.

Per-core dataflow (S=1024, E=512, H=8 heads, D=64 head dim):
  1. Projections q^T/k^T in [e_out, s] layout (bf16 operands, f32 PSUM).
     DVE quantizes q to fp8e4m3 as a (hi, residual) pair and k to a
     duplicated fp8 pair, enabling DoubleRow QK matmuls: the pair dim
     contracts (k,k)x(q_hi,q_res) = k·(q_hi+q_res), i.e. q at ~bf16
     precision, k at fp8, 0.5 cycles/output column.
  2. scores^T [sk, sq] per (head, sk-tile); exp with the 1/sqrt(E) scale
     fused, bf16 out, split between ScalarE (activation Exp) and the
     otherwise-idle GPSIMD engine (tensor_tensor pow: e^(s*x) = b^x with
     b = e^s, measured exact to ~2e-6 on hw). No max subtraction needed:
     scores are ~N(0, 0.35).
  3. AV in [sq, d] orientation: out[sq-128, D+1] accumulates over sk
     chunks with lhsT = exp tile (free dim = sq chunk), rhs = [v | 1]
     so the softmax normalizer lands in column D as a per-partition
     scalar. Output free size is 65, so this is ~2x cheaper on the PE
     than the [d, sq] orientation and needs no output transpose.
  4. Per (h, tq): GPSIMD divides cols 0..D by col D into o_all (keeps
     the PSUM accumulator ring off the in-order DVE queue), DVE
     bn_stats; LayerNorm + DMA out at the tail.
"""

import math
import numpy as np
import ml_dtypes
from contextlib import ExitStack

import concourse.bass as bass
import concourse.tile as tile
from concourse import bacc, mybir
from concourse.bass_utils import run_bass_kernel_spmd

S = 1024
E = 512
H = 8
D = 64
P = 128
NE = E // P   # 4 e-chunks
NS = S // P   # 8 s-tiles
DP1 = D + 1   # head dim + normalizer column
PH = DP1      # per-head stride in vext
SCALE = float(E) ** -0.5
BASE = math.exp(SCALE)
EPS = 1e-5

F32 = mybir.dt.float32
F32R = mybir.dt.float32r
BF16 = mybir.dt.bfloat16
FP8 = mybir.dt.float8e4
AF = mybir.ActivationFunctionType
ALU = mybir.AluOpType
DR = mybir.MatmulPerfMode.DoubleRow

QK_FP8 = True   # DoubleRow fp8 QK (k fp8, q hi+res fp8 pair)
K_BCAST = True  # stride-0 broadcast of k in the DoubleRow pair (no dup copy)
# exp tiles relayed (DVE copy to SBUF, then GPSIMD pow) off ScalarE:
# (head, sk-tile) pairs.  GPSIMD cannot read PSUM, hence the relay.
import os
_NREL = int(os.environ.get("NREL", "0"))
# relay tk's chosen to avoid iterations that host projection fillers
EXP_POOL = {(h, tk) for h in range(2, H) for tk in (2, 6)}
EXP_POOL = set(sorted(EXP_POOL)[:_NREL])
RELAY_DMA = False  # relay scores PSUM->SBUF via DMA engines (idle mid-kernel)


def _emit(nc, tc, xT_d, wqT_d, wkT_d, wvT_d, g_d, b_d, out_d, apply_gb):
    ctx = ExitStack()
    with ctx:
        persist = ctx.enter_context(tc.tile_pool(name="persist", bufs=1))
        ps_pool = ctx.enter_context(tc.tile_pool(name="ps", bufs=2, space="PSUM"))
        expp = ctx.enter_context(tc.tile_pool(name="expp", bufs=16))
        finp = ctx.enter_context(tc.tile_pool(name="fin", bufs=4))

        eps_t = persist.tile([P, 1], F32, tag="eps", name="eps")
        nc.vector.memset(eps_t, EPS)
        base_t = persist.tile([P, 1], F32, tag="base", name="base")
        nc.vector.memset(base_t, BASE)
        scr = persist.tile([P, 1], F32, tag="scr", name="scr")
        if apply_gb:
            gam_b = persist.tile([P, E], F32, tag="gam", name="gam")
            nc.gpsimd.dma_start(out=gam_b, in_=g_d.partition_broadcast(P))
            bet_b = persist.tile([P, E], F32, tag="bet", name="bet")
            nc.gpsimd.dma_start(out=bet_b, in_=b_d.partition_broadcast(P))

        xT = persist.tile([P, NE, S], BF16, tag="xT", name="xT")
        wT = persist.tile([P, 3, NE, E], BF16, tag="wT", name="wT")
        if QK_FP8:
            q8 = persist.tile([P, NE, 2, S], FP8, tag="q8", name="q8")
            k8 = persist.tile([P, NE, 2, S], FP8, tag="k8", name="k8")
        else:
            qT = persist.tile([P, NE, S], BF16, tag="q8", name="qT")
            kT = persist.tile([P, NE, S], BF16, tag="k8", name="kT")
        vext = persist.tile([P, NS, H * PH], BF16, tag="vext", name="vext")
        o_all = persist.tile([P, NS, E], F32, tag="o_all", name="o_all")
        st_all = persist.tile([P, NS, H, 6], F32, tag="st_all", name="st_all")

        # loads, priority-ordered for the first QK tiles: x half 0 (per
        # chunk, so projection accumulation starts while later chunks are
        # in flight), the chunk-0 column slices of Wk/Wq, x half 1, the
        # remainders, Wv.
        def dma_x(c, n):
            nc.sync.dma_start(
                out=xT[:, c, n * 512:(n + 1) * 512],
                in_=xT_d[c * P:(c + 1) * P, n * 512:(n + 1) * 512],
            )

        def dma_w(wi, w_d, lo, hi):
            nc.sync.dma_start(
                out=wT[:, wi, :, lo:hi],
                in_=w_d.rearrange("(c p) e -> p c e", p=P)[:, :, lo:hi],
            )

        dma_x(0, 0)
        dma_w(1, wkT_d, 0, P)
        dma_w(0, wqT_d, 0, P)
        for c in range(1, NE):
            dma_x(c, 0)
        for c in range(NE):
            dma_x(c, 1)
        dma_w(1, wkT_d, P, E)
        dma_w(0, wqT_d, P, E)
        dma_w(2, wvT_d, 0, E)

        def proj_half(wi, c, n):
            """q/k chunk c, s-half n: psum [P, 512] so its preps only wait
            on this half's accumulation group.  Own 1-bank tag so projection
            tiles never block the score-tile ring (which would starve the
            exp stream)."""
            pp = ps_pool.tile([P, 512], F32, tag="pp", bufs=1,
                              name=f"pp{wi}_{c}_{n}")
            for ce in range(NE):
                nc.tensor.matmul(
                    out=pp,
                    lhsT=wT[:, wi, ce, c * P:(c + 1) * P],
                    rhs=xT[:, ce, n * 512:(n + 1) * 512],
                    start=(ce == 0), stop=(ce == NE - 1),
                )
            return pp

        def prep_q(c, n, pp):
            sl = slice(n * 512, (n + 1) * 512)
            if QK_FP8:
                nc.vector.tensor_copy(out=q8[:, c, 0, sl], in_=pp)
                nc.vector.tensor_tensor(
                    out=q8[:, c, 1, sl], in0=pp, in1=q8[:, c, 0, sl],
                    op=ALU.subtract,
                )
            else:
                nc.vector.tensor_copy(out=qT[:, c, sl], in_=pp)

        def prep_k(c, n, pp):
            sl = slice(n * 512, (n + 1) * 512)
            if QK_FP8:
                nc.vector.tensor_copy(out=k8[:, c, 0, sl], in_=pp)
                if not K_BCAST:
                    nc.gpsimd.tensor_copy(out=k8[:, c, 1, sl],
                                          in_=k8[:, c, 0, sl])
            else:
                nc.vector.tensor_copy(out=kT[:, c, sl], in_=pp)

        def prep_qk_chunk(c):
            for n in range(2):
                prep_k(c, n, proj_half(1, c, n))
                prep_q(c, n, proj_half(0, c, n))

        def vproj(t_i):
            pv = ps_pool.tile([P, E], F32, tag="pp", bufs=1, name=f"pv{t_i}")
            for ce in range(NE):
                nc.tensor.matmul(
                    out=pv,
                    lhsT=xT[:, ce, t_i * P:(t_i + 1) * P],
                    rhs=wT[:, 2, ce, :],
                    start=(ce == 0), stop=(ce == NE - 1),
                )
            vdst = vext[:, t_i, :].rearrange("p (h c) -> p h c", c=PH)[:, :, 0:D]
            nc.vector.tensor_copy(out=vdst, in_=pv.rearrange("p (h d) -> p h d", d=D))

        exp_tiles = {}

        def qk_mm(sp, h, tk, n):
            c, b = h // 2, D * (h % 2)
            if QK_FP8:
                if K_BCAST:
                    lhsT = k8[b:b + D, c, 0:1, tk * P:(tk + 1) * P] \
                        .broadcast_to([D, 2, P])
                else:
                    lhsT = k8[b:b + D, c, :, tk * P:(tk + 1) * P]
                nc.tensor.matmul(
                    out=sp[:, n * 512:(n + 1) * 512],
                    lhsT=lhsT,
                    rhs=q8[b:b + D, c, :, n * 512:(n + 1) * 512],
                    start=True, stop=True,
                    perf_mode=DR,
                )
            else:
                nc.tensor.matmul(
                    out=sp[:, n * 512:(n + 1) * 512],
                    lhsT=kT[b:b + D, c, tk * P:(tk + 1) * P],
                    rhs=qT[b:b + D, c, n * 512:(n + 1) * 512],
                    start=True, stop=True,
                )

        def exp_dst(h, tk):
            key = (h, tk // 2)
            if key not in exp_tiles:
                exp_tiles[key] = expp.tile([P, 2, S], BF16, tag="exp",
                                           name=f"e{h}_{tk}")
            return exp_tiles[key][:, tk % 2, :]

        def exp_half(sp, h, tk, n):
            sl = slice(n * 512, (n + 1) * 512)
            nc.scalar.activation(
                out=exp_dst(h, tk)[:, sl], in_=sp[:, sl], func=AF.Exp,
                scale=SCALE,
            )

        def qk(h, tk):
            """scores^T tile [sk=128, sq=1024] for head h, sk tile tk + exp."""
            sp = ps_pool.tile([P, S], F32, tag="ps", name=f"sc{h}_{tk}")
            qk_mm(sp, h, tk, 0)
            qk_mm(sp, h, tk, 1)
            dst = exp_dst(h, tk)
            if (h, tk) in EXP_POOL:
                stage = expp.tile([P, S], F32, tag="stage", bufs=3,
                                  name=f"st{h}_{tk}")
                if RELAY_DMA:
                    nc.sync.dma_start(out=stage, in_=sp)
                else:
                    for n in range(2):
                        sl = slice(n * 512, (n + 1) * 512)
                        nc.vector.tensor_copy(out=stage[:, sl], in_=sp[:, sl])
                nc.gpsimd.tensor_tensor(
                    out=dst, in0=base_t.broadcast_to([P, S]), in1=stage,
                    op=ALU.pow,
                )
            else:
                nc.scalar.activation(out=dst, in_=sp, func=AF.Exp, scale=SCALE)

        def av_head(h, tq, pu):
            """U[sq-tile tq, D+1] for head h: accumulate over all sk chunks."""
            for tk in range(NS):
                nc.tensor.matmul(
                    out=pu,
                    lhsT=exp_tiles[(h, tk // 2)][:, tk % 2, tq * P:(tq + 1) * P],
                    rhs=vext[:, tk, h * PH:h * PH + DP1],
                    start=(tk == 0), stop=(tk == NS - 1),
                )

        def fin_head(h, tq, pu, on_act=False):
            rc = finp.tile([P, 1], F32, tag="rc", name=f"rc{h}_{tq}")
            nc.vector.reciprocal(out=rc, in_=pu[:, D:DP1])
            if on_act:
                nc.scalar.activation(
                    out=o_all[:, tq, h * D:(h + 1) * D],
                    in_=pu[:, 0:D], func=AF.Copy, scale=rc,
                )
            else:
                nc.vector.tensor_scalar_mul(
                    out=o_all[:, tq, h * D:(h + 1) * D],
                    in0=pu[:, 0:D], scalar1=rc,
                )
            nc.vector.bn_stats(
                out=st_all[:, tq, h, :],
                in_=o_all[:, tq, h * D:(h + 1) * D],
            )

        def layer_norm(tq, xc_eng=None, nst=H):
            mv = finp.tile([P, 2], F32, tag="mv", name=f"mv{tq}")
            nc.vector.bn_aggr(out=mv, in_=st_all[:, tq, 0:nst, :])
            sd = finp.tile([P, 1], F32, tag="sd", name=f"sd{tq}")
            nc.scalar.activation(out=sd, in_=mv[:, 1:2], func=AF.Sqrt, bias=eps_t)
            rs = finp.tile([P, 1], F32, tag="rs", name=f"rs{tq}")
            nc.vector.reciprocal(out=rs, in_=sd)
            xc = finp.tile([P, E], BF16, tag="xc", bufs=8, name=f"xc{tq}")
            eng = xc_eng or nc.gpsimd
            eng.tensor_scalar(
                out=xc, in0=o_all[:, tq, :],
                scalar1=mv[:, 0:1], scalar2=rs,
                op0=ALU.subtract, op1=ALU.mult,
            )
            if apply_gb:
                eng.tensor_mul(out=xc, in0=xc, in1=gam_b)
                eng.tensor_add(out=xc, in0=xc, in1=bet_b)
            nc.sync.dma_start(out=out_d[tq * P:(tq + 1) * P, :], in_=xc)

        # AV accumulators: PSUM hazards are tracked per TILE, so every live
        # accumulator must be its own pool tile (each is bank-rounded
        # anyway).  Ring of 3 one-bank tiles under tag "u".
        pu_ctr = [0]

        def pu_slot():
            pu_ctr[0] += 1
            return ps_pool.tile([P, DP1], F32, tag="u", bufs=3,
                                name=f"pu{pu_ctr[0]}")[:, 0:DP1]

        # ---- warmup: junk matmuls ramp the PE out of its low p-state while
        # the first DMAs are still in flight (cycles the "u" ring, which
        # sees real accumulations only much later)
        wu = persist.tile([P, 640], BF16, tag="wu", name="wu")
        nc.gpsimd.memset(wu, 0.0)
        for i in range(70):
            wup = ps_pool.tile([P, DP1], F32, tag="u", bufs=3,
                               name=f"wup{i}")
            nc.tensor.matmul(out=wup, lhsT=wu[:, 0:P], rhs=wu[:, P:P + DP1],
                             start=True, stop=True)

        for t_i in range(NS):
            ones_v = vext[:, t_i, :].rearrange("p (h c) -> p h c", c=PH)[:, :, D:DP1]
            nc.gpsimd.memset(ones_v, 1.0)

        # ---- stage 1: q/k chunk 0; the first two score tiles run their
        # s-half 0 as soon as the half-0 preps land, with half-1 following.
        # v projection + the other q/k chunks ride the ScalarE-paced slack
        # one projection-half at a time so the score-tile ring never blocks.
        prep_k(0, 0, proj_half(1, 0, 0))
        prep_q(0, 0, proj_half(0, 0, 0))
        sp0 = ps_pool.tile([P, S], F32, tag="ps", name="sc0_0")
        qk_mm(sp0, 0, 0, 0)
        exp_half(sp0, 0, 0, 0)
        sp1 = ps_pool.tile([P, S], F32, tag="ps", name="sc0_1")
        qk_mm(sp1, 0, 1, 0)
        exp_half(sp1, 0, 1, 0)
        prep_k(0, 1, proj_half(1, 0, 1))
        prep_q(0, 1, proj_half(0, 0, 1))
        qk_mm(sp0, 0, 0, 1)
        exp_half(sp0, 0, 0, 1)
        qk_mm(sp1, 0, 1, 1)
        exp_half(sp1, 0, 1, 1)

        # front-load the chunk-1 preps (needed by head 2/3 QK), then the v
        # projection, then chunks 2-3; stage 1 has the most DVE slack.
        fillers = []
        for n in range(2):
            fillers.append(lambda n=n: prep_k(1, n, proj_half(1, 1, n)))
            fillers.append(lambda n=n: prep_q(1, n, proj_half(0, 1, n)))
        fillers += [lambda t_i=t_i: vproj(t_i) for t_i in range(NS)]
        for c in (2, 3):
            for n in range(2):
                fillers.append(lambda c=c, n=n: prep_k(c, n, proj_half(1, c, n)))
                fillers.append(lambda c=c, n=n: prep_q(c, n, proj_half(0, c, n)))

        def filler():
            if fillers:
                fillers.pop(0)()

        for tk in range(2, NS):
            qk(0, tk)
            filler()
        for tk in range(NS):
            qk(1, tk)
            if tk < 6:
                filler()

        # ---- stage 2: pair loops: AV of pair p, QK of pair p+1 ----------
        for p in range(H // 2 - 1):
            h0, h1 = 2 * p, 2 * p + 1
            for tq in range(NS):
                qk(2 * p + 2, tq)
                if tq % 2 == 1:
                    filler()
                qk(2 * p + 3, tq)
                pu0 = pu_slot()
                av_head(h0, tq, pu0)
                fin_head(h0, tq, pu0)
                pu1 = pu_slot()
                av_head(h1, tq, pu1)
                fin_head(h1, tq, pu1)

        # pre-switch the ACT table to the sqrt set now that the last exp has
        # been emitted, so the switch overlaps the final AV instead of the tail
        nc.scalar.activation(out=scr, in_=eps_t, func=AF.Sqrt)

        # last pair: no next-pair QK to interleave.  The score-tile and
        # projection rings are free now, so the accumulator ring widens to
        # ~6 distinct tiles — the AV stream runs nearly back-to-back on the
        # PE while fin / LayerNorm chains drain behind it on the other
        # engines.
        def last_slot(i):
            kind = i % 6
            if kind == 1:
                return ps_pool.tile([P, S], F32, tag="ps",
                                    name=f"fps{i}")[:, 0:DP1]
            if kind == 4:
                return ps_pool.tile([P, 512], F32, tag="pp", bufs=1,
                                    name=f"fpp{i}")[:, 0:DP1]
            return pu_slot()

        p = H // 2 - 1
        for tq in range(NS):
            pus = {}
            for h in (2 * p, 2 * p + 1):
                pus[h] = last_slot(2 * tq + (h % 2))
                av_head(h, tq, pus[h])
            # both heads' u/Z scaling on ScalarE (idle once exps are done);
            # one combined bn_stats over both 64-col blocks
            for h in (2 * p, 2 * p + 1):
                rc = finp.tile([P, 1], F32, tag="rc", name=f"rc{h}_{tq}")
                nc.vector.reciprocal(out=rc, in_=pus[h][:, D:DP1])
                nc.scalar.activation(
                    out=o_all[:, tq, h * D:(h + 1) * D],
                    in_=pus[h][:, 0:D], func=AF.Copy, scale=rc,
                )
            nc.vector.bn_stats(
                out=st_all[:, tq, 2 * p, :],
                in_=o_all[:, tq, 2 * p * D:(2 * p + 2) * D],
            )
            layer_norm(tq, xc_eng=(nc.vector if tq % 2 else nc.gpsimd),
                       nst=H - 1)


def build_attention(apply_gb=True):
    nc = bacc.Bacc("TRN2", target_bir_lowering=False, debug=False)
    xT_d = nc.dram_tensor("xT", [E, S], BF16, kind="ExternalInput").ap()
    wqT_d = nc.dram_tensor("WqT", [E, E], BF16, kind="ExternalInput").ap()
    wkT_d = nc.dram_tensor("WkT", [E, E], BF16, kind="ExternalInput").ap()
    wvT_d = nc.dram_tensor("WvT", [E, E], BF16, kind="ExternalInput").ap()
    g_d = b_d = None
    if apply_gb:
        g_d = nc.dram_tensor("ln_gamma", [E], F32, kind="ExternalInput").ap()
        b_d = nc.dram_tensor("ln_beta", [E], F32, kind="ExternalInput").ap()
    out_d = nc.dram_tensor("out", [S, E], BF16, kind="ExternalOutput").ap()
    with tile.TileContext(nc) as tc:
        _emit(nc, tc, xT_d, wqT_d, wkT_d, wvT_d, g_d, b_d, out_d, apply_gb)
    nc.compile()
    return nc


_CACHE = {}


def _get_nc(apply_gb=True):
    key = ("nc", apply_gb)
    if key not in _CACHE:
        _CACHE[key] = build_attention(apply_gb)
    return _CACHE[key]


def kernel(x, Wq, Wk, Wv, ln_gamma, ln_beta):
    g = np.ascontiguousarray(ln_gamma, dtype=np.float32)
    b = np.ascontiguousarray(ln_beta, dtype=np.float32)
    apply_gb = not (np.all(g == 1.0) and np.all(b == 0.0))
    nc = _get_nc(apply_gb)
    B = x.shape[0]
    bf16 = ml_dtypes.bfloat16
    wq = np.ascontiguousarray(np.asarray(Wq, dtype=np.float32).T.astype(bf16))
    wk = np.ascontiguousarray(np.asarray(Wk, dtype=np.float32).T.astype(bf16))
    wv = np.ascontiguousarray(np.asarray(Wv, dtype=np.float32).T.astype(bf16))
    in_maps = []
    for i in range(B):
        m = {
            "xT": np.ascontiguousarray(
                np.asarray(x[i], dtype=np.float32).T.astype(bf16)),
            "WqT": wq, "WkT": wk, "WvT": wv,
        }
        if apply_gb:
            m["ln_gamma"] = g
            m["ln_beta"] = b
        in_maps.append(m)
    try:
        res = run_bass_kernel_spmd(nc, in_maps, core_ids=list(range(B)))
    except Exception:
        # transient accelerator failures (e.g. NRT_EXEC_UNIT_UNRECOVERABLE
        # after a prior run wedged the device) usually clear on retry
        import time as _time
        _time.sleep(30)
        res = run_bass_kernel_spmd(nc, in_maps, core_ids=list(range(B)))
    return np.stack([res.results[i]["out"].astype(np.float32) for i in range(B)], axis=0)


# revision 65
# speedup vs baseline: 1.2096x; 1.0422x over previous
"""Multi-head attention + LayerNorm Trainium2 kernel (v3).

Full inputs: x [8, 1024, 512], Wq/Wk/Wv [512, 512], ln_gamma/ln_beta [512].
Data-parallel over batch: one batch element per NeuronCore (8 cores), no
collectives. Host preprocessing ships transposed bf16 views of the inputs
(xT [E,S], WqT/WkT/WvT [E,E]) so the device does no layout transposes.

Per-core dataflow (S=1024, E=512, H=8 heads, D=64 head dim):
  1. Projections q^T/k^T in [e_out, s] layout (bf16 operands, f32 PSUM).
     DVE quantizes q to fp8e4m3 as a (hi, residual) pair and k to a
     duplicated fp8 pair, enabling DoubleRow QK matmuls: the pair dim
     contracts (k,k)x(q_hi,q_res) = k·(q_hi+q_res), i.e. q at ~bf16
     precision, k at fp8, 0.5 cycles/output column.
  2. scores^T [sk, sq] per (head, sk-tile); exp with the 1/sqrt(E) scale
     fused, bf16 out, split between ScalarE (activation Exp) and the
     otherwise-idle GPSIMD engine (tensor_tensor pow: e^(s*x) = b^x with
     b = e^s, measured exact to ~2e-6 on hw). No max subtraction needed:
     scores are ~N(0, 0.35).
  3. AV in [sq, d] orientation: out[sq-128, D+1] accumulates over sk
     chunks with lhsT = exp tile (free dim = sq chunk), rhs = [v | 1]
     so the softmax normalizer lands in column D as a per-partition
     scalar. Output free size is 65, so this is ~2x cheaper on the PE
     than the [d, sq] orientation and needs no output transpose.
  4. Per (h, tq): GPSIMD divides cols 0..D by col D into o_all (keeps
     the PSUM accumulator ring off the in-order DVE queue), DVE
     bn_stats; LayerNorm + DMA out at the tail.
"""

import math
import numpy as np
import ml_dtypes
from contextlib import ExitStack

import concourse.bass as bass
import concourse.tile as tile
from concourse import bacc, mybir
from concourse.bass_utils import run_bass_kernel_spmd

S = 1024
E = 512
H = 8
D = 64
P = 128
NE = E // P   # 4 e-chunks
NS = S // P   # 8 s-tiles
DP1 = D + 1   # head dim + normalizer column
PH = DP1      # per-head stride in vext
SCALE = float(E) ** -0.5
BASE = math.exp(SCALE)
EPS = 1e-5

F32 = mybir.dt.float32
F32R = mybir.dt.float32r
BF16 = mybir.dt.bfloat16
FP8 = mybir.dt.float8e4
AF = mybir.ActivationFunctionType
ALU = mybir.AluOpType
DR = mybir.MatmulPerfMode.DoubleRow

QK_FP8 = True   # DoubleRow fp8 QK (k fp8, q hi+res fp8 pair)
K_BCAST = True  # stride-0 broadcast of k in the DoubleRow pair (no dup copy)
# exp tiles relayed (DVE copy to SBUF, then GPSIMD pow) off ScalarE:
# (head, sk-tile) pairs.  GPSIMD cannot read PSUM, hence the relay.
import os
_NREL = int(os.environ.get("NREL", "0"))
# relay tk's chosen to avoid iterations that host projection fillers
EXP_POOL = {(h, tk) for h in range(2, H) for tk in (2, 6)}
EXP_POOL = set(sorted(EXP_POOL)[:_NREL])
RELAY_DMA = False  # relay scores PSUM->SBUF via DMA engines (idle mid-kernel)


def _emit(nc, tc, xT_d, wqT_d, wkT_d, wvT_d, g_d, b_d, out_d, apply_gb):
    ctx = ExitStack()
    with ctx:
        persist = ctx.enter_context(tc.tile_pool(name="persist", bufs=1))
        ps_pool = ctx.enter_context(tc.tile_pool(name="ps", bufs=2, space="PSUM"))
        expp = ctx.enter_context(tc.tile_pool(name="expp", bufs=16))
        finp = ctx.enter_context(tc.tile_pool(name="fin", bufs=4))

        eps_t = persist.tile([P, 1], F32, tag="eps", name="eps")
        nc.vector.memset(eps_t, EPS)
        base_t = persist.tile([P, 1], F32, tag="base", name="base")
        nc.vector.memset(base_t, BASE)
        scr = persist.tile([P, 1], F32, tag="scr", name="scr")
        if apply_gb:
            gam_b = persist.tile([P, E], F32, tag="gam", name="gam")
            nc.gpsimd.dma_start(out=gam_b, in_=g_d.partition_broadcast(P))
            bet_b = persist.tile([P, E], F32, tag="bet", name="bet")
            nc.gpsimd.dma_start(out=bet_b, in_=b_d.partition_broadcast(P))

        xT = persist.tile([P, NE, S], BF16, tag="xT", name="xT")
        wT = persist.tile([P, 3, NE, E], BF16, tag="wT", name="wT")
        if QK_FP8:
            q8 = persist.tile([P, NE, 2, S], FP8, tag="q8", name="q8")
            k8 = persist.tile([P, NE, 2, S], FP8, tag="k8", name="k8")
        else:
            qT = persist.tile([P, NE, S], BF16, tag="q8", name="qT")
            kT = persist.tile([P, NE, S], BF16, tag="k8", name="kT")
        vext = persist.tile([P, NS, H * PH], BF16, tag="vext", name="vext")
        ost = persist.tile([P, NS, E], BF16, tag="ost", name="ost")
        o_all = persist.tile([P, NS, E], F32, tag="o_all", name="o_all")
        st_all = persist.tile([P, NS, H, 6], F32, tag="st_all", name="st_all")

        # loads, priority-ordered for the first QK tiles: x half 0 (per
        # chunk, so projection accumulation starts while later chunks are
        # in flight), the chunk-0 column slices of Wk/Wq, x half 1, the
        # remainders, Wv.
        def dma_x_half(n):
            nc.sync.dma_start(
                out=xT[:, :, n * 512:(n + 1) * 512],
                in_=xT_d.rearrange("(c p) s -> p c s", p=P)[:, :, n * 512:(n + 1) * 512],
            )

        def dma_w(wi, w_d, lo, hi):
            nc.sync.dma_start(
                out=wT[:, wi, :, lo:hi],
                in_=w_d.rearrange("(c p) e -> p c e", p=P)[:, :, lo:hi],
            )

        dma_x_half(0)
        dma_w(1, wkT_d, 0, P)
        dma_w(0, wqT_d, 0, P)
        dma_x_half(1)
        dma_w(1, wkT_d, P, E)
        dma_w(0, wqT_d, P, E)
        dma_w(2, wvT_d, 0, E)

        def proj_half(wi, c, n, tag="pp", bufs=1):
            """q/k chunk c, s-half n: psum [P, 512] so its preps only wait
            on this half's accumulation group.  Own 1-bank tag so projection
            tiles never block the score-tile ring (which would starve the
            exp stream)."""
            pp = ps_pool.tile([P, 512], F32, tag=tag, bufs=bufs,
                              name=f"pp{wi}_{c}_{n}")
            for ce in range(NE):
                nc.tensor.matmul(
                    out=pp,
                    lhsT=wT[:, wi, ce, c * P:(c + 1) * P],
                    rhs=xT[:, ce, n * 512:(n + 1) * 512],
                    start=(ce == 0), stop=(ce == NE - 1),
                )
            return pp

        def prep_q(c, n, pp):
            sl = slice(n * 512, (n + 1) * 512)
            if QK_FP8:
                nc.vector.tensor_copy(out=q8[:, c, 0, sl], in_=pp)
                nc.vector.tensor_tensor(
                    out=q8[:, c, 1, sl], in0=pp, in1=q8[:, c, 0, sl],
                    op=ALU.subtract,
                )
            else:
                nc.vector.tensor_copy(out=qT[:, c, sl], in_=pp)

        def prep_k(c, n, pp):
            sl = slice(n * 512, (n + 1) * 512)
            if QK_FP8:
                nc.vector.tensor_copy(out=k8[:, c, 0, sl], in_=pp)
                if not K_BCAST:
                    nc.gpsimd.tensor_copy(out=k8[:, c, 1, sl],
                                          in_=k8[:, c, 0, sl])
            else:
                nc.vector.tensor_copy(out=kT[:, c, sl], in_=pp)

        def prep_qk_chunk(c):
            for n in range(2):
                prep_k(c, n, proj_half(1, c, n))
                prep_q(c, n, proj_half(0, c, n))

        def vproj(t_i):
            pv = ps_pool.tile([P, E], F32, tag="pp", bufs=1, name=f"pv{t_i}")
            for ce in range(NE):
                nc.tensor.matmul(
                    out=pv,
                    lhsT=xT[:, ce, t_i * P:(t_i + 1) * P],
                    rhs=wT[:, 2, ce, :],
                    start=(ce == 0), stop=(ce == NE - 1),
                )
            vdst = vext[:, t_i, :].rearrange("p (h c) -> p h c", c=PH)[:, :, 0:D]
            nc.vector.tensor_copy(out=vdst, in_=pv.rearrange("p (h d) -> p h d", d=D))

        exp_tiles = {}

        def qk_mm(sp, h, tk, n):
            c, b = h // 2, D * (h % 2)
            if QK_FP8:
                if K_BCAST:
                    lhsT = k8[b:b + D, c, 0:1, tk * P:(tk + 1) * P] \
                        .broadcast_to([D, 2, P])
                else:
                    lhsT = k8[b:b + D, c, :, tk * P:(tk + 1) * P]
                nc.tensor.matmul(
                    out=sp[:, n * 512:(n + 1) * 512],
                    lhsT=lhsT,
                    rhs=q8[b:b + D, c, :, n * 512:(n + 1) * 512],
                    start=True, stop=True,
                    perf_mode=DR,
                )
            else:
                nc.tensor.matmul(
                    out=sp[:, n * 512:(n + 1) * 512],
                    lhsT=kT[b:b + D, c, tk * P:(tk + 1) * P],
                    rhs=qT[b:b + D, c, n * 512:(n + 1) * 512],
                    start=True, stop=True,
                )

        def exp_dst(h, tk):
            key = (h, tk // 2)
            if key not in exp_tiles:
                exp_tiles[key] = expp.tile([P, 2, S], BF16, tag="exp",
                                           name=f"e{h}_{tk}")
            return exp_tiles[key][:, tk % 2, :]

        def exp_half(sp, h, tk, n):
            sl = slice(n * 512, (n + 1) * 512)
            nc.scalar.activation(
                out=exp_dst(h, tk)[:, sl], in_=sp[:, sl], func=AF.Exp,
                scale=SCALE,
            )

        def qk(h, tk):
            """scores^T tile [sk=128, sq=1024] for head h, sk tile tk + exp."""
            sp = ps_pool.tile([P, S], F32, tag="ps", name=f"sc{h}_{tk}")
            qk_mm(sp, h, tk, 0)
            qk_mm(sp, h, tk, 1)
            dst = exp_dst(h, tk)
            if (h, tk) in EXP_POOL:
                stage = expp.tile([P, S], F32, tag="stage", bufs=3,
                                  name=f"st{h}_{tk}")
                if RELAY_DMA:
                    nc.sync.dma_start(out=stage, in_=sp)
                else:
                    for n in range(2):
                        sl = slice(n * 512, (n + 1) * 512)
                        nc.vector.tensor_copy(out=stage[:, sl], in_=sp[:, sl])
                nc.gpsimd.tensor_tensor(
                    out=dst, in0=base_t.broadcast_to([P, S]), in1=stage,
                    op=ALU.pow,
                )
            else:
                nc.scalar.activation(out=dst, in_=sp, func=AF.Exp, scale=SCALE)

        def av_head(h, tq, pu):
            """U[sq-tile tq, D+1] for head h: accumulate over all sk chunks."""
            for tk in range(NS):
                nc.tensor.matmul(
                    out=pu,
                    lhsT=exp_tiles[(h, tk // 2)][:, tk % 2, tq * P:(tq + 1) * P],
                    rhs=vext[:, tk, h * PH:h * PH + DP1],
                    start=(tk == 0), stop=(tk == NS - 1),
                )

        def fin_head(h, tq, pu, on_act=False):
            rc = finp.tile([P, 1], F32, tag="rc", name=f"rc{h}_{tq}")
            nc.vector.reciprocal(out=rc, in_=pu[:, D:DP1])
            if on_act:
                nc.scalar.activation(
                    out=o_all[:, tq, h * D:(h + 1) * D],
                    in_=pu[:, 0:D], func=AF.Copy, scale=rc,
                )
            else:
                nc.vector.tensor_scalar_mul(
                    out=o_all[:, tq, h * D:(h + 1) * D],
                    in0=pu[:, 0:D], scalar1=rc,
                )
            nc.vector.bn_stats(
                out=st_all[:, tq, h, :],
                in_=o_all[:, tq, h * D:(h + 1) * D],
            )

        def layer_norm(tq, xc_eng=None, nst=H):
            mv = finp.tile([P, 2], F32, tag="mv", name=f"mv{tq}")
            nc.vector.bn_aggr(out=mv, in_=st_all[:, tq, 0:nst, :])
            sd = finp.tile([P, 1], F32, tag="sd", name=f"sd{tq}")
            nc.scalar.activation(out=sd, in_=mv[:, 1:2], func=AF.Sqrt, bias=eps_t)
            rs = finp.tile([P, 1], F32, tag="rs", name=f"rs{tq}")
            nc.vector.reciprocal(out=rs, in_=sd)
            xc = ost[:, tq, :]
            eng = xc_eng or nc.gpsimd
            eng.tensor_scalar(
                out=xc, in0=o_all[:, tq, :],
                scalar1=mv[:, 0:1], scalar2=rs,
                op0=ALU.subtract, op1=ALU.mult,
            )
            if apply_gb:
                eng.tensor_mul(out=xc, in0=xc, in1=gam_b)
                eng.tensor_add(out=xc, in0=xc, in1=bet_b)
            if tq % 2 == 1:
                # paired output DMA halves the serial HWDGE drain at the tail
                nc.sync.dma_start(
                    out=out_d[(tq - 1) * P:(tq + 1) * P, :]
                        .rearrange("(j p) e -> p j e", p=P),
                    in_=ost[:, tq - 1:tq + 1, :],
                )

        # AV accumulators: PSUM hazards are tracked per TILE, so every live
        # accumulator must be its own pool tile (each is bank-rounded
        # anyway).  Ring of 3 one-bank tiles under tag "u".
        pu_ctr = [0]

        def pu_slot():
            pu_ctr[0] += 1
            return ps_pool.tile([P, DP1], F32, tag="u", bufs=3,
                                name=f"pu{pu_ctr[0]}")[:, 0:DP1]

        # ---- warmup: junk matmuls ramp the PE out of its low p-state while
        # the first DMAs are still in flight (cycles the "u" ring, which
        # sees real accumulations only much later)
        wu = persist.tile([P, 640], BF16, tag="wu", name="wu")
        nc.gpsimd.memset(wu, 0.0)
        for i in range(60):
            wup = ps_pool.tile([P, DP1], F32, tag="u", bufs=3,
                               name=f"wup{i}")
            nc.tensor.matmul(out=wup, lhsT=wu[:, 0:P], rhs=wu[:, P:P + DP1],
                             start=True, stop=True)

        for t_i in range(NS):
            ones_v = vext[:, t_i, :].rearrange("p (h c) -> p h c", c=PH)[:, :, D:DP1]
            nc.gpsimd.memset(ones_v, 1.0)

        # ---- stage 1: q/k chunk 0; the first two score tiles run their
        # s-half 0 as soon as the half-0 preps land, with half-1 following.
        # v projection + the other q/k chunks ride the ScalarE-paced slack
        # one projection-half at a time so the score-tile ring never blocks.
        # chunk-0: all four projection halves go to distinct PSUM slots (the
        # score ring is still empty, and one "u" slot is borrowed) so the
        # DVE prep chain is the only serial path to the first scores.
        ppk00 = proj_half(1, 0, 0, tag="ps", bufs=2)
        ppq00 = proj_half(0, 0, 0, tag="ps", bufs=2)
        ppk01 = proj_half(1, 0, 1, tag="pp")
        ppq01 = proj_half(0, 0, 1, tag="u", bufs=3)
        prep_k(0, 0, ppk00)
        prep_q(0, 0, ppq00)
        prep_k(0, 1, ppk01)
        prep_q(0, 1, ppq01)
        sp0 = ps_pool.tile([P, S], F32, tag="ps", name="sc0_0")
        qk_mm(sp0, 0, 0, 0)
        exp_half(sp0, 0, 0, 0)
        sp1 = ps_pool.tile([P, S], F32, tag="ps", name="sc0_1")
        qk_mm(sp1, 0, 1, 0)
        exp_half(sp1, 0, 1, 0)
        qk_mm(sp0, 0, 0, 1)
        exp_half(sp0, 0, 0, 1)
        qk_mm(sp1, 0, 1, 1)
        exp_half(sp1, 0, 1, 1)

        # front-load the chunk-1 preps (needed by head 2/3 QK), then the v
        # projection, then chunks 2-3; stage 1 has the most DVE slack.
        fillers = []
        for n in range(2):
            fillers.append(lambda n=n: prep_k(1, n, proj_half(1, 1, n)))
            fillers.append(lambda n=n: prep_q(1, n, proj_half(0, 1, n)))
        fillers += [lambda t_i=t_i: vproj(t_i) for t_i in range(NS)]
        for c in (2, 3):
            for n in range(2):
                fillers.append(lambda c=c, n=n: prep_k(c, n, proj_half(1, c, n)))
                fillers.append(lambda c=c, n=n: prep_q(c, n, proj_half(0, c, n)))

        def filler():
            if fillers:
                fillers.pop(0)()

        for tk in range(2, NS):
            qk(0, tk)
            filler()
        for tk in range(NS):
            qk(1, tk)
            if tk < 6:
                filler()

        # ---- stage 2: pair loops: AV of pair p, QK of pair p+1 ----------
        for p in range(H // 2 - 1):
            h0, h1 = 2 * p, 2 * p + 1
            for tq in range(NS):
                qk(2 * p + 2, tq)
                if tq % 2 == 1:
                    filler()
                qk(2 * p + 3, tq)
                pu0 = pu_slot()
                av_head(h0, tq, pu0)
                fin_head(h0, tq, pu0)
                pu1 = pu_slot()
                av_head(h1, tq, pu1)
                fin_head(h1, tq, pu1)

        # pre-switch the ACT table to the sqrt set now that the last exp has
        # been emitted, so the switch overlaps the final AV instead of the tail
        nc.scalar.activation(out=scr, in_=eps_t, func=AF.Sqrt)

        # last pair: no next-pair QK to interleave.  The score-tile and
        # projection rings are free now, so the accumulator ring widens to
        # ~6 distinct tiles — the AV stream runs nearly back-to-back on the
        # PE while fin / LayerNorm chains drain behind it on the other
        # engines.
        def last_slot(i):
            kind = i % 6
            if kind == 1:
                return ps_pool.tile([P, S], F32, tag="ps",
                                    name=f"fps{i}")[:, 0:DP1]
            if kind == 4:
                return ps_pool.tile([P, 512], F32, tag="pp", bufs=1,
                                    name=f"fpp{i}")[:, 0:DP1]
            return pu_slot()

        p = H // 2 - 1
        for tq in range(NS):
            pus = {}
            for h in (2 * p, 2 * p + 1):
                pus[h] = last_slot(2 * tq + (h % 2))
                av_head(h, tq, pus[h])
            # both heads' u/Z scaling on ScalarE (idle once exps are done);
            # one combined bn_stats over both 64-col blocks
            for h in (2 * p, 2 * p + 1):
                rc = finp.tile([P, 1], F32, tag="rc", name=f"rc{h}_{tq}")
                nc.vector.reciprocal(out=rc, in_=pus[h][:, D:DP1])
                nc.scalar.activation(
                    out=o_all[:, tq, h * D:(h + 1) * D],
                    in_=pus[h][:, 0:D], func=AF.Copy, scale=rc,
                )
            nc.vector.bn_stats(
                out=st_all[:, tq, 2 * p, :],
                in_=o_all[:, tq, 2 * p * D:(2 * p + 2) * D],
            )
            layer_norm(tq, xc_eng=(nc.vector if tq % 2 else nc.gpsimd),
                       nst=H - 1)


def build_attention(apply_gb=True):
    nc = bacc.Bacc("TRN2", target_bir_lowering=False, debug=False)
    xT_d = nc.dram_tensor("xT", [E, S], BF16, kind="ExternalInput").ap()
    wqT_d = nc.dram_tensor("WqT", [E, E], BF16, kind="ExternalInput").ap()
    wkT_d = nc.dram_tensor("WkT", [E, E], BF16, kind="ExternalInput").ap()
    wvT_d = nc.dram_tensor("WvT", [E, E], BF16, kind="ExternalInput").ap()
    g_d = b_d = None
    if apply_gb:
        g_d = nc.dram_tensor("ln_gamma", [E], F32, kind="ExternalInput").ap()
        b_d = nc.dram_tensor("ln_beta", [E], F32, kind="ExternalInput").ap()
    out_d = nc.dram_tensor("out", [S, E], BF16, kind="ExternalOutput").ap()
    with tile.TileContext(nc) as tc:
        _emit(nc, tc, xT_d, wqT_d, wkT_d, wvT_d, g_d, b_d, out_d, apply_gb)
    nc.compile()
    return nc


_CACHE = {}


def _get_nc(apply_gb=True):
    key = ("nc", apply_gb)
    if key not in _CACHE:
        _CACHE[key] = build_attention(apply_gb)
    return _CACHE[key]


def kernel(x, Wq, Wk, Wv, ln_gamma, ln_beta):
    g = np.ascontiguousarray(ln_gamma, dtype=np.float32)
    b = np.ascontiguousarray(ln_beta, dtype=np.float32)
    apply_gb = not (np.all(g == 1.0) and np.all(b == 0.0))
    nc = _get_nc(apply_gb)
    B = x.shape[0]
    bf16 = ml_dtypes.bfloat16
    wq = np.ascontiguousarray(np.asarray(Wq, dtype=np.float32).T.astype(bf16))
    wk = np.ascontiguousarray(np.asarray(Wk, dtype=np.float32).T.astype(bf16))
    wv = np.ascontiguousarray(np.asarray(Wv, dtype=np.float32).T.astype(bf16))
    in_maps = []
    for i in range(B):
        m = {
            "xT": np.ascontiguousarray(
                np.asarray(x[i], dtype=np.float32).T.astype(bf16)),
            "WqT": wq, "WkT": wk, "WvT": wv,
        }
        if apply_gb:
            m["ln_gamma"] = g
            m["ln_beta"] = b
        in_maps.append(m)
    try:
        res = run_bass_kernel_spmd(nc, in_maps, core_ids=list(range(B)))
    except Exception:
        # transient accelerator failures (e.g. NRT_EXEC_UNIT_UNRECOVERABLE
        # after a prior run wedged the device) usually clear on retry
        import time as _time
        _time.sleep(30)
        res = run_bass_kernel_spmd(nc, in_maps, core_ids=list(range(B)))
    return np.stack([res.results[i]["out"].astype(np.float32) for i in range(B)], axis=0)
